# revision 1
# baseline (speedup 1.0000x reference)
"""Trainium2 Bass kernel for GNN message passing:

    messages = e @ W_e.T + (h @ W_hu.T)[src] + (h @ W_hw.T)[tgt]

Strategy (8 NeuronCores, edge-parallel):
  - Edges are sharded 100k per core; h and the three weight matrices are
    replicated.
  - Phase 1 (per core): project the full node table once,
    hu = h @ W_hu.T and hw = h @ W_hw.T, into internal DRAM tables.
  - Phase 2 (per core): 2048-edge groups. ee = e @ W_e.T on the tensor
    engine into PSUM; hu[src] / hw[tgt] fetched with the DMAGatherAnt
    instruction (2048 rows per call, descriptors generated by the Q7
    CounterMachine); DVE sums psum + hu_g + hw_g; one DMA stores the
    group.
  - DMAGatherAnt indices are int16, so the node table is addressed in
    two halves (split at row 32768). The host permutes each core's
    edges into 4 buckets by (src-half, tgt-half); each bucket is padded
    to a fixed capacity so the SPMD program is identical on all cores.
    The host applies the inverse permutation when assembling the output.

Host-side prep only reshapes/permutes inputs and un-permutes the
output; all FLOPs and gathers run on device in fp32.
"""
import os
from contextlib import ExitStack

import numpy as np

import concourse.bass as bass
import concourse.tile as tile
from concourse import bacc, mybir
from concourse.bass_utils import run_bass_kernel_spmd

N_NODES = 50000
N_EDGES = 800000
IN_DIM = 128
OUT_DIM = 128
EDGE_DIM = 64
NCORES = 8

P = 128
NODES_PAD = 50176           # 98 * 512 = 392 * 128
HALF = 24576                # table split: lo tables ready ~49% into phase 1
HI_ROWS = NODES_PAD - HALF  # 17408

EPC = N_EDGES // NCORES     # 100000 edges per core
G_EDGES = 2048              # edges per gather group
# Bucket capacities in groups of 2048 edges, bucket = 2*(src>=HALF)+(tgt>=HALF).
# Means for uniform random indices: ~[24159, 24993, 24993, 25855] edges
# -> generous (>5 sigma) fixed capacities so the SPMD program is uniform.
CAPS = [13, 13, 13, 14]
NGRP = sum(CAPS)            # 53 groups
EPC_PAD = NGRP * G_EDGES    # 108544
SEG_EDGE_START = [0]
for _c in CAPS:
    SEG_EDGE_START.append(SEG_EDGE_START[-1] + _c * G_EDGES)

F32 = mybir.dt.float32
I16 = mybir.dt.int16

_CACHE = {}
LAST = {}


def _build():
    nc = bacc.Bacc(
        "TRN2",
        target_bir_lowering=False,
        debug=False,
        enable_asserts=True,
        num_devices=NCORES,
        num_swdge_queues=2,
    )

    hT = nc.dram_tensor("hT", [P, NODES_PAD], F32, kind="ExternalInput").ap()
    Wcat = nc.dram_tensor("Wcat", [P, 2 * OUT_DIM], F32, kind="ExternalInput").ap()
    WeT = nc.dram_tensor("WeT", [2 * EDGE_DIM, OUT_DIM], F32, kind="ExternalInput").ap()
    eP = nc.dram_tensor("eP", [NGRP, P, G_EDGES // 2], F32, kind="ExternalInput").ap()
    sidx = nc.dram_tensor("sidx", [P, NGRP * (G_EDGES // 16)], I16, kind="ExternalInput").ap()
    tidx = nc.dram_tensor("tidx", [P, NGRP * (G_EDGES // 16)], I16, kind="ExternalInput").ap()
    msgs = nc.dram_tensor("msgs", [EPC_PAD, OUT_DIM], F32, kind="ExternalOutput").ap()

    # lo/hi are separate tensors so phase-2 lo-segment gathers only depend
    # on the lo-half table writes and can overlap the rest of phase 1.
    hu_lo = nc.dram_tensor("hu_lo", [HALF, OUT_DIM], F32).ap()
    hu_hi = nc.dram_tensor("hu_hi", [HI_ROWS, OUT_DIM], F32).ap()
    hw_lo = nc.dram_tensor("hw_lo", [HALF, OUT_DIM], F32).ap()
    hw_hi = nc.dram_tensor("hw_hi", [HI_ROWS, OUT_DIM], F32).ap()

    with tile.TileContext(nc) as tc:
        with ExitStack() as ctx:
            wpool = ctx.enter_context(tc.tile_pool(name="w", bufs=1))
            idxpool = ctx.enter_context(tc.tile_pool(name="idx", bufs=1))
            hpool = ctx.enter_context(tc.tile_pool(name="h", bufs=3))
            p1psum = ctx.enter_context(tc.tile_pool(name="p1psum", bufs=4, space="PSUM"))
            opool = ctx.enter_context(tc.tile_pool(name="o", bufs=3))
            epool = ctx.enter_context(tc.tile_pool(name="e", bufs=3))
            p2psum = ctx.enter_context(tc.tile_pool(name="p2psum", bufs=2, space="PSUM"))
            gpool = ctx.enter_context(tc.tile_pool(name="g", bufs=6))
            mpool = ctx.enter_context(tc.tile_pool(name="m", bufs=3))

            wcat_t = wpool.tile([P, 2 * OUT_DIM], F32)
            nc.scalar.dma_start(out=wcat_t[:], in_=Wcat[:])
            wet_t = wpool.tile([2 * EDGE_DIM, OUT_DIM], F32)
            nc.scalar.dma_start(out=wet_t[:], in_=WeT[:])
            sidx_t = idxpool.tile([P, NGRP * (G_EDGES // 16)], I16)
            nc.sync.dma_start(out=sidx_t[:], in_=sidx[:])
            tidx_t = idxpool.tile([P, NGRP * (G_EDGES // 16)], I16)
            nc.sync.dma_start(out=tidx_t[:], in_=tidx[:])

            # ---- Phase 1: hu/hw node tables -------------------------------
            # 512 nodes per block: 4 matmuls into one 2-bank PSUM tile, one
            # wide DVE copy, then one batched store per table.
            for i in range(NODES_PAD // 512):
                hb = hpool.tile([P, 512], F32)
                nc.scalar.dma_start(out=hb[:], in_=hT[:, i * 512 : (i + 1) * 512])
                ot = opool.tile([P, 1024], F32)
                for half in range(2):
                    ps = p1psum.tile([P, 512], F32)
                    for s in range(2):
                        nc.tensor.matmul(
                            out=ps[:, s * 256 : (s + 1) * 256],
                            lhsT=hb[:, (half * 2 + s) * P : (half * 2 + s + 1) * P],
                            rhs=wcat_t[:],
                            start=True,
                            stop=True,
                        )
                    nc.vector.tensor_copy(out=ot[:, half * 512 : (half + 1) * 512], in_=ps[:])
                ot3 = ot[:].rearrange("p (s x) -> p s x", s=4)
                if i < HALF // 512:
                    hu_dst, hw_dst, n0 = hu_lo, hw_lo, i * 512
                else:
                    hu_dst, hw_dst, n0 = hu_hi, hw_hi, i * 512 - HALF
                nc.sync.dma_start(
                    out=hu_dst[n0 : n0 + 512, :].rearrange("(s p) d -> p s d", p=P),
                    in_=ot3[:, :, 0:OUT_DIM],
                )
                nc.scalar.dma_start(
                    out=hw_dst[n0 : n0 + 512, :].rearrange("(s p) d -> p s d", p=P),
                    in_=ot3[:, :, OUT_DIM : 2 * OUT_DIM],
                )

            # ---- Phase 2: per-edge messages, 4 bucket segments ------------
            G = 0
            for seg in range(4):
                hu_src = (hu_lo if seg < 2 else hu_hi)[:]
                hw_src = (hw_lo if seg % 2 == 0 else hw_hi)[:]
                for _ in range(CAPS[seg]):
                    eb = epool.tile([P, G_EDGES // 2], F32)
                    nc.scalar.dma_start(out=eb[:], in_=eP[G])

                    hu_g = gpool.tile([P, G_EDGES], F32, tag="hu_g")
                    nc.gpsimd.dma_gather(
                        out_ap=hu_g[:].rearrange("p (c d) -> p c d", c=16),
                        in_ap=hu_src,
                        idxs_ap=sidx_t[:, G * 128 : (G + 1) * 128],
                        num_idxs=G_EDGES,
                        num_idxs_reg=G_EDGES,
                        elem_size=OUT_DIM,
                        single_packet=False,
                        queue_num=0,
                    )
                    hw_g = gpool.tile([P, G_EDGES], F32, tag="hw_g")
                    nc.gpsimd.dma_gather(
                        out_ap=hw_g[:].rearrange("p (c d) -> p c d", c=16),
                        in_ap=hw_src,
                        idxs_ap=tidx_t[:, G * 128 : (G + 1) * 128],
                        num_idxs=G_EDGES,
                        num_idxs_reg=G_EDGES,
                        elem_size=OUT_DIM,
                        single_packet=False,
                        queue_num=1,
                    )

                    mt = mpool.tile([P, G_EDGES], F32)
                    for hhalf in range(2):
                        ps = p2psum.tile([P, 1024], F32)
                        for q in range(8):
                            t = hhalf * 8 + q
                            pb = 0 if t < 8 else EDGE_DIM
                            nc.tensor.matmul(
                                out=ps[:, q * P : (q + 1) * P],
                                lhsT=eb[pb : pb + EDGE_DIM, (t % 8) * P : (t % 8 + 1) * P],
                                rhs=wet_t[pb : pb + EDGE_DIM, :],
                                start=True,
                                stop=True,
                            )
                        sl = slice(hhalf * 1024, (hhalf + 1) * 1024)
                        nc.vector.tensor_add(out=mt[:, sl], in0=ps[:], in1=hu_g[:, sl])
                        nc.vector.tensor_add(out=mt[:, sl], in0=mt[:, sl], in1=hw_g[:, sl])

                    nc.sync.dma_start(
                        out=msgs[G * G_EDGES : (G + 1) * G_EDGES, :].rearrange(
                            "(t p) d -> p t d", p=P
                        ),
                        in_=mt[:].rearrange("p (t d) -> p t d", t=16),
                    )
                    G += 1

    nc.compile()
    return nc


def get_nc():
    if "nc" not in _CACHE:
        _CACHE["nc"] = _build()
    return _CACHE["nc"]


def _prep_in_maps(h, e, edge_index, W_e, W_hu, W_hw):
    """Returns (in_maps, pos_list): pos_list[c][i] = row of core c's device
    output holding original edge c*EPC+i."""
    h = np.ascontiguousarray(np.asarray(h, dtype=np.float32))
    e = np.ascontiguousarray(np.asarray(e, dtype=np.float32))
    src = np.asarray(edge_index[0]).astype(np.int64)
    tgt = np.asarray(edge_index[1]).astype(np.int64)
    W_e = np.asarray(W_e, dtype=np.float32)
    W_hu = np.asarray(W_hu, dtype=np.float32)
    W_hw = np.asarray(W_hw, dtype=np.float32)

    hT = np.zeros((P, NODES_PAD), dtype=np.float32)
    hT[:, :N_NODES] = h.T
    Wcat = np.ascontiguousarray(np.concatenate([W_hu.T, W_hw.T], axis=1))
    # stacked twice so phase 2 has a copy at SBUF base partition 0 and 64
    WeT = np.ascontiguousarray(np.vstack([W_e.T, W_e.T]))

    in_maps = []
    pos_list = []
    for c in range(NCORES):
        sl = slice(c * EPC, (c + 1) * EPC)
        sc, tc_, ec = src[sl], tgt[sl], e[sl]
        bucket = 2 * (sc >= HALF).astype(np.int64) + (tc_ >= HALF).astype(np.int64)

        e_pad = np.zeros((EPC_PAD, EDGE_DIM), dtype=np.float32)
        s16 = np.zeros((EPC_PAD,), dtype=np.int16)
        t16 = np.zeros((EPC_PAD,), dtype=np.int16)
        pos = np.empty((EPC,), dtype=np.int64)
        for b in range(4):
            selb = np.flatnonzero(bucket == b)
            if len(selb) > CAPS[b] * G_EDGES:
                raise RuntimeError(
                    f"bucket {b} overflow on core {c}: {len(selb)} > {CAPS[b] * G_EDGES}"
                )
            base = SEG_EDGE_START[b]
            pos[selb] = base + np.arange(len(selb))
            e_pad[base : base + len(selb)] = ec[selb]
            s16[base : base + len(selb)] = (sc[selb] - HALF * (b >> 1)).astype(np.int16)
            t16[base : base + len(selb)] = (tc_[selb] - HALF * (b & 1)).astype(np.int16)

        ePc = np.ascontiguousarray(
            e_pad.reshape(NGRP, 2, G_EDGES // 2, EDGE_DIM).transpose(0, 1, 3, 2)
        ).reshape(NGRP, P, G_EDGES // 2)

        # dma_gather index layout: value j of group g sits at
        # [j % 16, g*128 + j//16], replicated across the 8 gpsimd banks.
        def idx_layout(v16):
            a16 = v16.reshape(NGRP, G_EDGES // 16, 16).transpose(2, 0, 1).reshape(
                16, NGRP * (G_EDGES // 16)
            )
            return np.ascontiguousarray(np.tile(a16, (8, 1)))

        in_maps.append(
            {
                "hT": hT,
                "Wcat": Wcat,
                "WeT": WeT,
                "eP": ePc,
                "sidx": idx_layout(s16),
                "tidx": idx_layout(t16),
            }
        )
        pos_list.append(pos)
    return in_maps, pos_list


def _install_ntff_hook():
    """Best-effort: register the axon NTFF profile hook when the image's
    antenv package lacks axon_hooks (needed only for trace=True runs)."""
    import sys
    import types

    try:
        from antenv.axon_hooks import get_axon_ntff_profile_hook  # noqa: F401

        return
    except ImportError:
        pass
    try:
        from trn_agent_boot.trn_boot import _ntff_profile_via_ctypes

        hook = _ntff_profile_via_ctypes("/opt/axon/libaxon_pjrt.so")
        mod = types.ModuleType("antenv.axon_hooks")
        mod._hook = hook
        mod.get_axon_ntff_profile_hook = lambda: mod._hook
        mod.set_axon_ntff_profile_hook = lambda h: setattr(mod, "_hook", h)
        sys.modules["antenv.axon_hooks"] = mod
        import antenv

        antenv.axon_hooks = mod
    except Exception:
        pass


def kernel(h, e, edge_index, W_e, W_hu, W_hw):
    nc = get_nc()
    in_maps, pos_list = _prep_in_maps(h, e, edge_index, W_e, W_hu, W_hw)
    trace = bool(int(os.environ.get("KERNEL_TRACE", "0")))
    if trace:
        _install_ntff_hook()
    res = run_bass_kernel_spmd(nc, in_maps, list(range(NCORES)), trace=trace)
    LAST["exec_time_ns"] = res.exec_time_ns
    LAST["results"] = res
    out = np.empty((N_EDGES, OUT_DIM), dtype=np.float32)
    for c in range(NCORES):
        out[c * EPC : (c + 1) * EPC] = res.results[c]["msgs"][pos_list[c]]
    return out



# revision 20
# speedup vs baseline: 1.4594x; 1.4594x over previous
"""Trainium2 Bass kernel for GNN message passing:

    messages = e @ W_e.T + (h @ W_hu.T)[src] + (h @ W_hw.T)[tgt]

Strategy (8 NeuronCores, edge-parallel, bf16, raw-bass manual pipeline):
  - Edges sharded 100k per core; h and weights replicated. All device
    math in bf16 (harness gate is 2e-2; this lands ~4e-3).
  - Phase 1: project the node table once into an internal-DRAM combined
    table hub[n] = [hu[n] | hw[n]] (bf16, 512B rows, so table-store DMAs
    move 512B contiguous segments at full rate). The lo half (25088
    rows) is written first so phase-2 gathers of lo/lo edges start while
    the hi half is still being built.
  - Phase 2: per 2048-edge group, hu[src] / hw[tgt] are fetched with
    non-transpose DMAGatherAnt (elem 256B, elem_step 512B into the two
    column halves of hub), spread over 4 SWDGE queues -- measured ~3.6x
    the 1-queue random-gather rate, and non-transpose mode is the only
    multi-queue-safe mode (concurrent transpose-mode gathers corrupt
    each other through the shared per-engine transpose path). The tensor
    engine computes ee = e.T @ W_e.T into PSUM (edges on partitions);
    DVE does mt = psum + hu_g + hw_g; one DMA stores each group.
  - Raw engine blocks with manual per-slot semaphores (the Tile
    scheduler's SWDGE-sem round-robin cannot express 4-queue gathers).
  - int16 gather indices cover 32k rows -> hub addressed as lo/hi halves
    (split 25088); the host buckets each core's edges by
    (src-half, tgt-half) into 4 fixed-capacity segments so the SPMD
    program is uniform, and un-permutes the output.
"""
import os
from contextlib import ExitStack

import numpy as np
import ml_dtypes

import concourse.bass as bass
from concourse import bacc, mybir
from concourse.bass_utils import run_bass_kernel_spmd
from concourse.library_config import mlp

N_NODES = 50000
N_EDGES = 800000
IN_DIM = 128
OUT_DIM = 128
EDGE_DIM = 64
NCORES = 8

P = 128
NODES_PAD = 50176
HALF = 25088                # int16 index range split
NBLK = NODES_PAD // 512     # 98 phase-1 blocks of 512 nodes
LOBLK = HALF // 512         # 49 blocks cover the lo half

EPC = N_EDGES // NCORES     # 100000 edges per core
G_EDGES = 2048              # edges per gather group
CAPS = [13, 13, 13, 13]     # groups per bucket; mean 25000 +- 137, cap 26624
NGRP = sum(CAPS)            # 52
EPC_PAD = NGRP * G_EDGES    # 106496
SEG_EDGE_START = [0]
for _c in CAPS:
    SEG_EDGE_START.append(SEG_EDGE_START[-1] + _c * G_EDGES)

HBUF = 3                    # phase-1 h-block buffers
GBUF = 8                    # gather group buffers (even: slot sems stay on
                            # one queue parity)
EBUF = 4                    # e-tile buffers
MBUF = 3                    # output tile buffers

F32 = mybir.dt.float32
BF16 = mybir.dt.bfloat16
I16 = mybir.dt.int16
NPBF16 = ml_dtypes.bfloat16

_CACHE = {}
LAST = {}


def _seg_of(G):
    s = 0
    while G >= sum(CAPS[: s + 1]):
        s += 1
    return s


def _build():
    nc = bacc.Bacc(
        "TRN2",
        target_bir_lowering=False,
        debug=False,
        enable_asserts=True,
        num_devices=NCORES,
        num_swdge_queues=4,
    )

    hT = nc.dram_tensor("hT", [P, NODES_PAD], BF16, kind="ExternalInput")
    # cols 0:256 = [W_hu.T | W_hw.T]; cols 256:384 = W_e.T stacked twice
    wall = nc.dram_tensor("wall", [P, 384], BF16, kind="ExternalInput")
    eP = nc.dram_tensor("eP", [NGRP, P, G_EDGES // 2], BF16, kind="ExternalInput")
    sidx = nc.dram_tensor("sidx", [P, NGRP * (G_EDGES // 16)], I16, kind="ExternalInput")
    tidx = nc.dram_tensor("tidx", [P, NGRP * (G_EDGES // 16)], I16, kind="ExternalInput")
    msgs = nc.dram_tensor("msgs", [NGRP, P, G_EDGES], BF16, kind="ExternalOutput")
    hub = nc.dram_tensor("hub", [NODES_PAD, 2 * OUT_DIM], BF16)

    with (
        nc.Block() as block,
        nc.sbuf_tensor("wall_t", [P, 384], BF16) as wall_t,
        nc.sbuf_tensor("sidx_t", [P, NGRP * (G_EDGES // 16)], I16) as sidx_t,
        nc.sbuf_tensor("tidx_t", [P, NGRP * (G_EDGES // 16)], I16) as tidx_t,
        ExitStack() as ctx,
    ):
        hb = [
            ctx.enter_context(nc.sbuf_tensor(f"hb{i}", [P, 512], BF16))
            for i in range(HBUF)
        ]
        ot = [
            ctx.enter_context(nc.sbuf_tensor(f"ot{i}", [P, 1024], BF16))
            for i in range(2)
        ]
        eb = [
            ctx.enter_context(nc.sbuf_tensor(f"eb{i}", [P, G_EDGES // 2], BF16))
            for i in range(EBUF)
        ]
        hug = [
            ctx.enter_context(nc.sbuf_tensor(f"hug{i}", [P, 16, OUT_DIM], BF16))
            for i in range(GBUF)
        ]
        hwg = [
            ctx.enter_context(nc.sbuf_tensor(f"hwg{i}", [P, 16, OUT_DIM], BF16))
            for i in range(GBUF)
        ]
        mtb = [
            ctx.enter_context(nc.sbuf_tensor(f"mt{i}", [P, G_EDGES], BF16))
            for i in range(MBUF)
        ]
        ps1 = [
            ctx.enter_context(nc.psum_tensor(f"ps1_{i}", [P, 512], F32))
            for i in range(4)
        ]
        ps2 = [
            ctx.enter_context(nc.psum_tensor(f"ps2_{i}", [P, 1024], F32))
            for i in range(2)
        ]
        s_ld = ctx.enter_context(nc.semaphore("s_ld"))
        s_hb = [ctx.enter_context(nc.semaphore(f"s_hb{i}")) for i in range(HBUF)]
        s_p1 = ctx.enter_context(nc.semaphore("s_p1"))
        s_c1d = ctx.enter_context(nc.semaphore("s_c1d"))
        s_c1a = ctx.enter_context(nc.semaphore("s_c1a"))
        s_t = [ctx.enter_context(nc.semaphore(f"s_t{i}")) for i in range(2)]
        s_eb = [ctx.enter_context(nc.semaphore(f"s_eb{i}")) for i in range(EBUF)]
        s_hs = [ctx.enter_context(nc.semaphore(f"s_hs{i}")) for i in range(GBUF)]
        s_ht = [ctx.enter_context(nc.semaphore(f"s_ht{i}")) for i in range(GBUF)]
        s_mm = ctx.enter_context(nc.semaphore("s_mm"))
        s_a1 = ctx.enter_context(nc.semaphore("s_a1"))
        s_add = ctx.enter_context(nc.semaphore("s_add"))
        s_st = [ctx.enter_context(nc.semaphore(f"s_st{i}")) for i in range(MBUF)]

        # table-store counts per slot once the lo half (blocks 0..48) is done
        LO_T = [16 * 25, 16 * 24]
        FULL_T = [16 * 49, 16 * 49]

        @block.sync
        def _(sp: bass.BassEngine):
            sp.dma_start(wall_t[:, :], wall[:, :]).then_inc(s_ld, 16)
            sp.dma_start(sidx_t[:, :], sidx[:, :]).then_inc(s_ld, 16)
            sp.dma_start(tidx_t[:, :], tidx[:, :]).then_inc(s_ld, 16)
            for b in range(NBLK):
                sp.wait_ge(s_c1d, b + 1)
                sp.wait_ge(s_c1a, b + 1)
                sp.dma_start(
                    hub[b * 512 : (b + 1) * 512, :].rearrange(
                        "(s p) d -> p s d", p=P
                    ),
                    ot[b % 2][:, :].rearrange("p (s d) -> p s d", s=4),
                ).then_inc(s_t[b % 2], 16)
            for G in range(NGRP):
                sp.wait_ge(s_add, 2 * G + 2)
                sp.dma_start(msgs[G], mtb[G % MBUF][:, :]).then_inc(
                    s_st[G % MBUF], 16
                )
            for k in range(MBUF):
                sp.wait_ge(s_st[k], 16 * ((NGRP - 1 - k) // MBUF + 1))

        @block.scalar
        def _(act: bass.BassScalarEngine):
            for i in range(HBUF):
                act.dma_start(hb[i][:, :], hT[:, i * 512 : (i + 1) * 512]).then_inc(
                    s_hb[i], 16
                )
            for b in range(NBLK):
                act.wait_ge(s_p1, 2 * b + 2)
                if b >= 2:
                    act.wait_ge(s_t[b % 2], 16 * (b // 2))
                act.copy(
                    out=ot[b % 2][:, 512:1024], in_=ps1[(2 * b + 1) % 4][:, :]
                ).then_inc(s_c1a, 1)
                if b + HBUF < NBLK:
                    act.dma_start(
                        hb[(b + HBUF) % HBUF][:, :],
                        hT[:, (b + HBUF) * 512 : (b + HBUF + 1) * 512],
                    ).then_inc(s_hb[(b + HBUF) % HBUF], 16)
            for i in range(EBUF):
                act.dma_start(eb[i][:, :], eP[i]).then_inc(s_eb[i], 16)
            for G in range(NGRP):
                if G + EBUF < NGRP:
                    act.wait_ge(s_mm, 2 * G + 2)
                    act.dma_start(
                        eb[(G + EBUF) % EBUF][:, :], eP[G + EBUF]
                    ).then_inc(s_eb[(G + EBUF) % EBUF], 16)

        @block.vector
        def _(dve: bass.BassVectorEngine):
            for b in range(NBLK):
                dve.wait_ge(s_p1, 2 * b + 1)
                if b >= 2:
                    dve.wait_ge(s_t[b % 2], 16 * (b // 2))
                dve.tensor_copy(
                    out=ot[b % 2][:, 0:512], in_=ps1[(2 * b) % 4][:, :]
                ).then_inc(s_c1d, 1)
            for G in range(NGRP):
                hu_t = hug[G % GBUF][:, :, :].rearrange("p c d -> p (c d)")
                hw_t = hwg[G % GBUF][:, :, :].rearrange("p c d -> p (c d)")
                dve.wait_ge(s_hs[G % GBUF], 16 * (G // GBUF + 1))
                dve.wait_ge(s_ht[G % GBUF], 16 * (G // GBUF + 1))
                if G >= MBUF:
                    dve.wait_ge(s_st[G % MBUF], 16 * ((G - MBUF) // MBUF + 1))
                mt = mtb[G % MBUF]
                dve.wait_ge(s_mm, 2 * G + 1)
                dve.tensor_add(
                    out=mt[:, 0:1024], in0=ps2[0][:, :], in1=hu_t[:, 0:1024]
                )
                dve.wait_ge(s_mm, 2 * G + 2)
                dve.tensor_add(
                    out=mt[:, 1024:2048], in0=ps2[1][:, :], in1=hu_t[:, 1024:2048]
                ).then_inc(s_a1, 1)
                # self-edge: DVE is pipelined; reading mt back requires the
                # adds above to have fully drained to SBUF
                dve.wait_ge(s_a1, G + 1)
                dve.tensor_add(
                    out=mt[:, 0:1024], in0=mt[:, 0:1024], in1=hw_t[:, 0:1024]
                )
                dve.tensor_add(
                    out=mt[:, 1024:2048],
                    in0=mt[:, 1024:2048],
                    in1=hw_t[:, 1024:2048],
                ).then_inc(s_add, 2)

        @block.gpsimd
        def _(gp: bass.BassGpSimd):
            gp.load_library(mlp)
            gp.wait_ge(s_ld, 48)
            for G in range(NGRP):
                seg = _seg_of(G)
                if G == 0:
                    gp.wait_ge(s_t[0], LO_T[0])
                    gp.wait_ge(s_t[1], LO_T[1])
                elif G == CAPS[0]:
                    gp.wait_ge(s_t[0], FULL_T[0])
                    gp.wait_ge(s_t[1], FULL_T[1])
                hu_src = (
                    hub[0:HALF, 0:OUT_DIM]
                    if seg < 2
                    else hub[HALF:NODES_PAD, 0:OUT_DIM]
                )
                hw_src = (
                    hub[0:HALF, OUT_DIM : 2 * OUT_DIM]
                    if seg % 2 == 0
                    else hub[HALF:NODES_PAD, OUT_DIM : 2 * OUT_DIM]
                )
                if G >= GBUF:
                    gp.wait_ge(s_add, 2 * (G - GBUF) + 2)
                gp.dma_gather(
                    hug[G % GBUF][:, :, :],
                    hu_src,
                    sidx_t[:, G * 128 : (G + 1) * 128],
                    G_EDGES,
                    G_EDGES,
                    OUT_DIM,
                    elem_step=2 * OUT_DIM,
                    single_packet=False,
                    queue_num=2 * (G % 2),
                ).then_inc(s_hs[G % GBUF], 16)
                gp.dma_gather(
                    hwg[G % GBUF][:, :, :],
                    hw_src,
                    tidx_t[:, G * 128 : (G + 1) * 128],
                    G_EDGES,
                    G_EDGES,
                    OUT_DIM,
                    elem_step=2 * OUT_DIM,
                    single_packet=False,
                    queue_num=2 * (G % 2) + 1,
                ).then_inc(s_ht[G % GBUF], 16)

        @block.tensor
        def _(pe: bass.BassTensorEngine):
            pe.wait_ge(s_ld, 48)
            for b in range(NBLK):
                pe.wait_ge(s_hb[b % HBUF], 16 * (b // HBUF + 1))
                if b >= 2:
                    pe.wait_ge(s_c1d, b - 1)
                    pe.wait_ge(s_c1a, b - 1)
                for h in range(2):
                    ps = ps1[(2 * b + h) % 4]
                    for s in range(2):
                        mm = pe.matmul(
                            out=ps[:, s * 256 : (s + 1) * 256],
                            lhsT=hb[b % HBUF][:, (2 * h + s) * P : (2 * h + s + 1) * P],
                            rhs=wall_t[:, 0:256],
                            start=True,
                            stop=True,
                        )
                    mm.then_inc(s_p1, 1)
            for G in range(NGRP):
                pe.wait_ge(s_eb[G % EBUF], 16 * (G // EBUF + 1))
                if G >= 1:
                    pe.wait_ge(s_a1, G)
                for h in range(2):
                    pb = 0 if h == 0 else 64
                    for t in range(8):
                        mm = pe.matmul(
                            out=ps2[h][:, t * P : (t + 1) * P],
                            lhsT=eb[G % EBUF][pb : pb + 64, t * P : (t + 1) * P],
                            rhs=wall_t[pb : pb + 64, 256:384],
                            start=True,
                            stop=True,
                        )
                    mm.then_inc(s_mm, 1)

    nc.compile()
    return nc


def get_nc():
    if "nc" not in _CACHE:
        _CACHE["nc"] = _build()
    return _CACHE["nc"]


def _prep_in_maps(h, e, edge_index, W_e, W_hu, W_hw):
    """Returns (in_maps, pos_list): pos_list[c][i] = padded-edge slot of
    core c holding original edge c*EPC+i (slot = g*2048 + c*128 + p)."""
    h = np.asarray(h, dtype=np.float32)
    e = np.asarray(e, dtype=np.float32)
    src = np.asarray(edge_index[0]).astype(np.int64)
    tgt = np.asarray(edge_index[1]).astype(np.int64)
    W_e = np.asarray(W_e, dtype=np.float32)
    W_hu = np.asarray(W_hu, dtype=np.float32)
    W_hw = np.asarray(W_hw, dtype=np.float32)

    hT = np.zeros((P, NODES_PAD), dtype=NPBF16)
    hT[:, :N_NODES] = h.astype(NPBF16).T

    wall = np.concatenate(
        [W_hu.T, W_hw.T, np.vstack([W_e.T, W_e.T])], axis=1
    ).astype(NPBF16)

    in_maps = []
    pos_list = []
    for c in range(NCORES):
        sl = slice(c * EPC, (c + 1) * EPC)
        sc, tc_, ec = src[sl], tgt[sl], e[sl]
        bucket = 2 * (sc >= HALF).astype(np.int64) + (tc_ >= HALF).astype(np.int64)

        e_pad = np.zeros((EPC_PAD, EDGE_DIM), dtype=np.float32)
        s16 = np.zeros((EPC_PAD,), dtype=np.int16)
        t16 = np.zeros((EPC_PAD,), dtype=np.int16)
        pos = np.empty((EPC,), dtype=np.int64)
        for b in range(4):
            selb = np.flatnonzero(bucket == b)
            if len(selb) > CAPS[b] * G_EDGES:
                raise RuntimeError(
                    f"bucket {b} overflow on core {c}: {len(selb)} > {CAPS[b] * G_EDGES}"
                )
            base = SEG_EDGE_START[b]
            pos[selb] = base + np.arange(len(selb))
            e_pad[base : base + len(selb)] = ec[selb]
            s16[base : base + len(selb)] = (sc[selb] - HALF * (b >> 1)).astype(np.int16)
            t16[base : base + len(selb)] = (tc_[selb] - HALF * (b & 1)).astype(np.int16)

        ePc = np.ascontiguousarray(
            e_pad.reshape(NGRP, 2, G_EDGES // 2, EDGE_DIM)
            .astype(NPBF16)
            .transpose(0, 1, 3, 2)
        ).reshape(NGRP, P, G_EDGES // 2)

        # dma_gather index layout: value j of group g sits at
        # [j % 16, g*128 + j//16], replicated across the 8 gpsimd banks.
        def idx_layout(v16):
            a16 = v16.reshape(NGRP, G_EDGES // 16, 16).transpose(2, 0, 1).reshape(
                16, NGRP * (G_EDGES // 16)
            )
            return np.ascontiguousarray(np.tile(a16, (8, 1)))

        in_maps.append(
            {
                "hT": hT,
                "wall": wall,
                "eP": ePc,
                "sidx": idx_layout(s16),
                "tidx": idx_layout(t16),
            }
        )
        pos_list.append(pos)
    return in_maps, pos_list


def _unscramble(m):
    """[NGRP, P, G_EDGES] device layout -> [EPC_PAD, OUT_DIM]; edge slot
    g*2048 + c*128 + p lives at m[g, p, c*128:(c+1)*128]."""
    m4 = np.asarray(m).reshape(NGRP, P, 16, OUT_DIM)
    return np.ascontiguousarray(m4.transpose(0, 2, 1, 3)).reshape(EPC_PAD, OUT_DIM)


def _install_ntff_hook():
    """Best-effort: register the axon NTFF profile hook when the image's
    antenv package lacks axon_hooks (needed only for trace=True runs)."""
    import sys
    import types

    try:
        from antenv.axon_hooks import get_axon_ntff_profile_hook  # noqa: F401

        return
    except ImportError:
        pass
    try:
        from trn_agent_boot.trn_boot import _ntff_profile_via_ctypes

        hook = _ntff_profile_via_ctypes("/opt/axon/libaxon_pjrt.so")
        mod = types.ModuleType("antenv.axon_hooks")
        mod._hook = hook
        mod.get_axon_ntff_profile_hook = lambda: mod._hook
        mod.set_axon_ntff_profile_hook = lambda h: setattr(mod, "_hook", h)
        sys.modules["antenv.axon_hooks"] = mod
        import antenv

        antenv.axon_hooks = mod
    except Exception:
        pass


def kernel(h, e, edge_index, W_e, W_hu, W_hw):
    nc = get_nc()
    in_maps, pos_list = _prep_in_maps(h, e, edge_index, W_e, W_hu, W_hw)
    trace = bool(int(os.environ.get("KERNEL_TRACE", "0")))
    if trace:
        _install_ntff_hook()
    res = run_bass_kernel_spmd(nc, in_maps, list(range(NCORES)), trace=trace)
    LAST["exec_time_ns"] = res.exec_time_ns
    LAST["results"] = res
    out = np.empty((N_EDGES, OUT_DIM), dtype=np.float32)
    for c in range(NCORES):
        flat = _unscramble(res.results[c]["msgs"])
        out[c * EPC : (c + 1) * EPC] = flat[pos_list[c]].astype(np.float32)
    return out


# revision 21
# speedup vs baseline: 1.5369x; 1.0532x over previous
"""Trainium2 Bass kernel for GNN message passing:

    messages = e @ W_e.T + (h @ W_hu.T)[src] + (h @ W_hw.T)[tgt]

Strategy (8 NeuronCores, edge-parallel, bf16, raw-bass manual pipeline):
  - Edges sharded 100k per core; h and weights replicated. All device
    math in bf16 (harness gate is 2e-2; this lands ~4e-3).
  - Phase 1: project the node table once into an internal-DRAM combined
    table hub[n] = [hu[n] | hw[n]] (bf16, 512B rows, so table-store DMAs
    move 512B contiguous segments at full rate). The lo half (25088
    rows) is written first so phase-2 gathers of lo/lo edges start while
    the hi half is still being built.
  - Phase 2: per 2048-edge group, hu[src] / hw[tgt] are fetched with
    non-transpose DMAGatherAnt (elem 256B, elem_step 512B into the two
    column halves of hub), spread over 4 SWDGE queues -- measured ~3.6x
    the 1-queue random-gather rate, and non-transpose mode is the only
    multi-queue-safe mode (concurrent transpose-mode gathers corrupt
    each other through the shared per-engine transpose path). The tensor
    engine computes ee = e.T @ W_e.T into PSUM (edges on partitions);
    DVE does mt = psum + hu_g + hw_g; one DMA stores each group.
  - Raw engine blocks with manual per-slot semaphores (the Tile
    scheduler's SWDGE-sem round-robin cannot express 4-queue gathers).
  - int16 gather indices cover 32k rows -> hub addressed as lo/hi halves
    (split 25088); the host buckets each core's edges by
    (src-half, tgt-half) into 4 fixed-capacity segments so the SPMD
    program is uniform, and un-permutes the output.
"""
import os
from contextlib import ExitStack

import numpy as np
import ml_dtypes

import concourse.bass as bass
from concourse import bacc, mybir
from concourse.bass_utils import run_bass_kernel_spmd
from concourse.library_config import mlp

N_NODES = 50000
N_EDGES = 800000
IN_DIM = 128
OUT_DIM = 128
EDGE_DIM = 64
NCORES = 8

P = 128
NODES_PAD = 50176
HALF = 25088                # int16 index range split
NBLK = NODES_PAD // 512     # 98 phase-1 blocks of 512 nodes
LOBLK = HALF // 512         # 49 blocks cover the lo half

EPC = N_EDGES // NCORES     # 100000 edges per core
G_EDGES = 2048              # edges per gather group
CAPS = [13, 13, 13, 13]     # groups per bucket; mean 25000 +- 137, cap 26624
NGRP = sum(CAPS)            # 52
EPC_PAD = NGRP * G_EDGES    # 106496
SEG_EDGE_START = [0]
for _c in CAPS:
    SEG_EDGE_START.append(SEG_EDGE_START[-1] + _c * G_EDGES)

HBUF = 3                    # phase-1 h-block buffers
GBUF = 8                    # gather group buffers (even: slot sems stay on
                            # one queue parity)
EBUF = 4                    # e-tile buffers
MBUF = 4                    # output tile buffers
BPS = 4                     # phase-1 blocks per table store

F32 = mybir.dt.float32
BF16 = mybir.dt.bfloat16
I16 = mybir.dt.int16
NPBF16 = ml_dtypes.bfloat16

_CACHE = {}
LAST = {}


def _seg_of(G):
    s = 0
    while G >= sum(CAPS[: s + 1]):
        s += 1
    return s


def _build():
    nc = bacc.Bacc(
        "TRN2",
        target_bir_lowering=False,
        debug=False,
        enable_asserts=True,
        num_devices=NCORES,
        num_swdge_queues=4,
    )

    hT = nc.dram_tensor("hT", [P, NODES_PAD], BF16, kind="ExternalInput")
    # cols 0:256 = [W_hu.T | W_hw.T]; cols 256:384 = W_e.T stacked twice
    wall = nc.dram_tensor("wall", [P, 384], BF16, kind="ExternalInput")
    eP = nc.dram_tensor("eP", [NGRP, P, G_EDGES // 2], BF16, kind="ExternalInput")
    sidx = nc.dram_tensor("sidx", [P, NGRP * (G_EDGES // 16)], I16, kind="ExternalInput")
    tidx = nc.dram_tensor("tidx", [P, NGRP * (G_EDGES // 16)], I16, kind="ExternalInput")
    msgs = nc.dram_tensor("msgs", [NGRP, P, G_EDGES], BF16, kind="ExternalOutput")
    hub = nc.dram_tensor("hub", [NODES_PAD, 2 * OUT_DIM], BF16)

    with (
        nc.Block(no_gpsimd_drain=True) as block,
        nc.sbuf_tensor("wall_t", [P, 384], BF16) as wall_t,
        nc.sbuf_tensor("sidx_t", [P, NGRP * (G_EDGES // 16)], I16) as sidx_t,
        nc.sbuf_tensor("tidx_t", [P, NGRP * (G_EDGES // 16)], I16) as tidx_t,
        ExitStack() as ctx,
    ):
        hb = [
            ctx.enter_context(nc.sbuf_tensor(f"hb{i}", [P, 512], BF16))
            for i in range(HBUF)
        ]
        ot = [
            ctx.enter_context(nc.sbuf_tensor(f"ot{i}", [P, BPS * 1024], BF16))
            for i in range(2)
        ]
        eb = [
            ctx.enter_context(nc.sbuf_tensor(f"eb{i}", [P, G_EDGES // 2], BF16))
            for i in range(EBUF)
        ]
        hug = [
            ctx.enter_context(nc.sbuf_tensor(f"hug{i}", [P, 16, OUT_DIM], BF16))
            for i in range(GBUF)
        ]
        hwg = [
            ctx.enter_context(nc.sbuf_tensor(f"hwg{i}", [P, 16, OUT_DIM], BF16))
            for i in range(GBUF)
        ]
        mtb = [
            ctx.enter_context(nc.sbuf_tensor(f"mt{i}", [P, G_EDGES], BF16))
            for i in range(MBUF)
        ]
        ps1 = [
            ctx.enter_context(nc.psum_tensor(f"ps1_{i}", [P, 512], F32))
            for i in range(4)
        ]
        ps2 = [
            ctx.enter_context(nc.psum_tensor(f"ps2_{i}", [P, 1024], F32))
            for i in range(2)
        ]
        s_ld = ctx.enter_context(nc.semaphore("s_ld"))
        s_hb = [ctx.enter_context(nc.semaphore(f"s_hb{i}")) for i in range(HBUF)]
        s_p1 = ctx.enter_context(nc.semaphore("s_p1"))
        s_c1d = ctx.enter_context(nc.semaphore("s_c1d"))
        s_c1a = ctx.enter_context(nc.semaphore("s_c1a"))
        s_t = [ctx.enter_context(nc.semaphore(f"s_t{i}")) for i in range(2)]
        s_eb = [ctx.enter_context(nc.semaphore(f"s_eb{i}")) for i in range(EBUF)]
        s_hs = [ctx.enter_context(nc.semaphore(f"s_hs{i}")) for i in range(GBUF)]
        s_ht = [ctx.enter_context(nc.semaphore(f"s_ht{i}")) for i in range(GBUF)]
        s_mm = ctx.enter_context(nc.semaphore("s_mm"))
        s_a1 = ctx.enter_context(nc.semaphore("s_a1"))
        s_add = ctx.enter_context(nc.semaphore("s_add"))
        s_st = [ctx.enter_context(nc.semaphore(f"s_st{i}")) for i in range(MBUF)]

        # store s covers blocks [s*BPS, (s+1)*BPS); lo rows need blocks 0..48
        # -> stores 0..12; full table -> stores 0..24 (NBLK=98, BPS=4 -> 25)
        NST = NBLK // BPS + (1 if NBLK % BPS else 0)
        LO_ST = (LOBLK + BPS - 1) // BPS  # 13
        LO_T = [16 * ((LO_ST - 1 - k) // 2 + 1) for k in range(2)]
        FULL_T = [16 * ((NST - 1 - k) // 2 + 1) for k in range(2)]

        @block.sync
        def _(sp: bass.BassEngine):
            sp.dma_start(wall_t[:, :], wall[:, :]).then_inc(s_ld, 16)
            sp.dma_start(sidx_t[:, :], sidx[:, :]).then_inc(s_ld, 16)
            sp.dma_start(tidx_t[:, :], tidx[:, :]).then_inc(s_ld, 16)
            for st in range(NBLK // BPS + (1 if NBLK % BPS else 0)):
                blo, bhi = st * BPS, min((st + 1) * BPS, NBLK)
                sp.wait_ge(s_c1d, bhi)
                sp.wait_ge(s_c1a, bhi)
                sp.dma_start(
                    hub[blo * 512 : bhi * 512, :].rearrange(
                        "(s p) d -> p s d", p=P
                    ),
                    ot[st % 2][:, : (bhi - blo) * 1024].rearrange(
                        "p (s d) -> p s d", d=2 * OUT_DIM
                    ),
                ).then_inc(s_t[st % 2], 16)
            for G in range(NGRP):
                sp.wait_ge(s_add, 2 * G + 2)
                sp.dma_start(msgs[G], mtb[G % MBUF][:, :]).then_inc(
                    s_st[G % MBUF], 16
                )
            for k in range(MBUF):
                sp.wait_ge(s_st[k], 16 * ((NGRP - 1 - k) // MBUF + 1))

        @block.scalar
        def _(act: bass.BassScalarEngine):
            for i in range(HBUF):
                act.dma_start(hb[i][:, :], hT[:, i * 512 : (i + 1) * 512]).then_inc(
                    s_hb[i], 16
                )
            for b in range(NBLK):
                act.wait_ge(s_p1, 2 * b + 2)
                st = b // BPS
                if st >= 2 and b % BPS == 0:
                    act.wait_ge(s_t[st % 2], 16 * (st // 2))
                off = (b % BPS) * 1024
                act.copy(
                    out=ot[st % 2][:, off + 512 : off + 1024],
                    in_=ps1[(2 * b + 1) % 4][:, :],
                ).then_inc(s_c1a, 1)
                if b + HBUF < NBLK:
                    act.dma_start(
                        hb[(b + HBUF) % HBUF][:, :],
                        hT[:, (b + HBUF) * 512 : (b + HBUF + 1) * 512],
                    ).then_inc(s_hb[(b + HBUF) % HBUF], 16)
            for i in range(EBUF):
                act.dma_start(eb[i][:, :], eP[i]).then_inc(s_eb[i], 16)
            for G in range(NGRP):
                if G + EBUF < NGRP:
                    act.wait_ge(s_mm, 2 * G + 2)
                    act.dma_start(
                        eb[(G + EBUF) % EBUF][:, :], eP[G + EBUF]
                    ).then_inc(s_eb[(G + EBUF) % EBUF], 16)

        @block.vector
        def _(dve: bass.BassVectorEngine):
            for b in range(NBLK):
                dve.wait_ge(s_p1, 2 * b + 1)
                st = b // BPS
                if st >= 2 and b % BPS == 0:
                    dve.wait_ge(s_t[st % 2], 16 * (st // 2))
                off = (b % BPS) * 1024
                dve.tensor_copy(
                    out=ot[st % 2][:, off : off + 512], in_=ps1[(2 * b) % 4][:, :]
                ).then_inc(s_c1d, 1)
            for G in range(NGRP):
                hu_t = hug[G % GBUF][:, :, :].rearrange("p c d -> p (c d)")
                hw_t = hwg[G % GBUF][:, :, :].rearrange("p c d -> p (c d)")
                dve.wait_ge(s_hs[G % GBUF], 16 * (G // GBUF + 1))
                dve.wait_ge(s_ht[G % GBUF], 16 * (G // GBUF + 1))
                if G >= MBUF:
                    dve.wait_ge(s_st[G % MBUF], 16 * ((G - MBUF) // MBUF + 1))
                mt = mtb[G % MBUF]
                dve.wait_ge(s_mm, 2 * G + 1)
                dve.tensor_add(
                    out=mt[:, 0:1024], in0=ps2[0][:, :], in1=hu_t[:, 0:1024]
                )
                dve.wait_ge(s_mm, 2 * G + 2)
                dve.tensor_add(
                    out=mt[:, 1024:2048], in0=ps2[1][:, :], in1=hu_t[:, 1024:2048]
                ).then_inc(s_a1, 1)
                # self-edge: DVE is pipelined; reading mt back requires the
                # adds above to have fully drained to SBUF
                dve.wait_ge(s_a1, G + 1)
                dve.tensor_add(
                    out=mt[:, 0:1024], in0=mt[:, 0:1024], in1=hw_t[:, 0:1024]
                )
                dve.tensor_add(
                    out=mt[:, 1024:2048],
                    in0=mt[:, 1024:2048],
                    in1=hw_t[:, 1024:2048],
                ).then_inc(s_add, 2)

        @block.gpsimd
        def _(gp: bass.BassGpSimd):
            gp.load_library(mlp)
            gp.wait_ge(s_ld, 48)
            for G in range(NGRP):
                seg = _seg_of(G)
                if G == 0:
                    gp.wait_ge(s_t[0], LO_T[0])
                    gp.wait_ge(s_t[1], LO_T[1])
                elif G == CAPS[0]:
                    gp.wait_ge(s_t[0], FULL_T[0])
                    gp.wait_ge(s_t[1], FULL_T[1])
                hu_src = (
                    hub[0:HALF, 0:OUT_DIM]
                    if seg < 2
                    else hub[HALF:NODES_PAD, 0:OUT_DIM]
                )
                hw_src = (
                    hub[0:HALF, OUT_DIM : 2 * OUT_DIM]
                    if seg % 2 == 0
                    else hub[HALF:NODES_PAD, OUT_DIM : 2 * OUT_DIM]
                )
                if G >= GBUF:
                    gp.wait_ge(s_add, 2 * (G - GBUF) + 2)
                gp.dma_gather(
                    hug[G % GBUF][:, :, :],
                    hu_src,
                    sidx_t[:, G * 128 : (G + 1) * 128],
                    G_EDGES,
                    G_EDGES,
                    OUT_DIM,
                    elem_step=2 * OUT_DIM,
                    single_packet=False,
                    queue_num=2 * (G % 2),
                ).then_inc(s_hs[G % GBUF], 16)
                gp.dma_gather(
                    hwg[G % GBUF][:, :, :],
                    hw_src,
                    tidx_t[:, G * 128 : (G + 1) * 128],
                    G_EDGES,
                    G_EDGES,
                    OUT_DIM,
                    elem_step=2 * OUT_DIM,
                    single_packet=False,
                    queue_num=2 * (G % 2) + 1,
                ).then_inc(s_ht[G % GBUF], 16)

        @block.tensor
        def _(pe: bass.BassTensorEngine):
            pe.wait_ge(s_ld, 48)
            for b in range(NBLK):
                pe.wait_ge(s_hb[b % HBUF], 16 * (b // HBUF + 1))
                if b >= 2:
                    pe.wait_ge(s_c1d, b - 1)
                    pe.wait_ge(s_c1a, b - 1)
                for h in range(2):
                    ps = ps1[(2 * b + h) % 4]
                    for s in range(2):
                        mm = pe.matmul(
                            out=ps[:, s * 256 : (s + 1) * 256],
                            lhsT=hb[b % HBUF][:, (2 * h + s) * P : (2 * h + s + 1) * P],
                            rhs=wall_t[:, 0:256],
                            start=True,
                            stop=True,
                        )
                    mm.then_inc(s_p1, 1)
            for G in range(NGRP):
                pe.wait_ge(s_eb[G % EBUF], 16 * (G // EBUF + 1))
                if G >= 1:
                    pe.wait_ge(s_a1, G)
                for h in range(2):
                    pb = 0 if h == 0 else 64
                    for t in range(8):
                        mm = pe.matmul(
                            out=ps2[h][:, t * P : (t + 1) * P],
                            lhsT=eb[G % EBUF][pb : pb + 64, t * P : (t + 1) * P],
                            rhs=wall_t[pb : pb + 64, 256:384],
                            start=True,
                            stop=True,
                        )
                    mm.then_inc(s_mm, 1)

    nc.compile()
    return nc


def get_nc():
    if "nc" not in _CACHE:
        _CACHE["nc"] = _build()
    return _CACHE["nc"]


def _prep_in_maps(h, e, edge_index, W_e, W_hu, W_hw):
    """Returns (in_maps, pos_list): pos_list[c][i] = padded-edge slot of
    core c holding original edge c*EPC+i (slot = g*2048 + c*128 + p)."""
    h = np.asarray(h, dtype=np.float32)
    e = np.asarray(e, dtype=np.float32)
    src = np.asarray(edge_index[0]).astype(np.int64)
    tgt = np.asarray(edge_index[1]).astype(np.int64)
    W_e = np.asarray(W_e, dtype=np.float32)
    W_hu = np.asarray(W_hu, dtype=np.float32)
    W_hw = np.asarray(W_hw, dtype=np.float32)

    hT = np.zeros((P, NODES_PAD), dtype=NPBF16)
    hT[:, :N_NODES] = h.astype(NPBF16).T

    wall = np.concatenate(
        [W_hu.T, W_hw.T, np.vstack([W_e.T, W_e.T])], axis=1
    ).astype(NPBF16)

    in_maps = []
    pos_list = []
    for c in range(NCORES):
        sl = slice(c * EPC, (c + 1) * EPC)
        sc, tc_, ec = src[sl], tgt[sl], e[sl]
        bucket = 2 * (sc >= HALF).astype(np.int64) + (tc_ >= HALF).astype(np.int64)

        e_pad = np.zeros((EPC_PAD, EDGE_DIM), dtype=np.float32)
        s16 = np.zeros((EPC_PAD,), dtype=np.int16)
        t16 = np.zeros((EPC_PAD,), dtype=np.int16)
        pos = np.empty((EPC,), dtype=np.int64)
        for b in range(4):
            selb = np.flatnonzero(bucket == b)
            if len(selb) > CAPS[b] * G_EDGES:
                raise RuntimeError(
                    f"bucket {b} overflow on core {c}: {len(selb)} > {CAPS[b] * G_EDGES}"
                )
            base = SEG_EDGE_START[b]
            pos[selb] = base + np.arange(len(selb))
            e_pad[base : base + len(selb)] = ec[selb]
            s16[base : base + len(selb)] = (sc[selb] - HALF * (b >> 1)).astype(np.int16)
            t16[base : base + len(selb)] = (tc_[selb] - HALF * (b & 1)).astype(np.int16)

        ePc = np.ascontiguousarray(
            e_pad.reshape(NGRP, 2, G_EDGES // 2, EDGE_DIM)
            .astype(NPBF16)
            .transpose(0, 1, 3, 2)
        ).reshape(NGRP, P, G_EDGES // 2)

        # dma_gather index layout: value j of group g sits at
        # [j % 16, g*128 + j//16], replicated across the 8 gpsimd banks.
        def idx_layout(v16):
            a16 = v16.reshape(NGRP, G_EDGES // 16, 16).transpose(2, 0, 1).reshape(
                16, NGRP * (G_EDGES // 16)
            )
            return np.ascontiguousarray(np.tile(a16, (8, 1)))

        in_maps.append(
            {
                "hT": hT,
                "wall": wall,
                "eP": ePc,
                "sidx": idx_layout(s16),
                "tidx": idx_layout(t16),
            }
        )
        pos_list.append(pos)
    return in_maps, pos_list


def _unscramble(m):
    """[NGRP, P, G_EDGES] device layout -> [EPC_PAD, OUT_DIM]; edge slot
    g*2048 + c*128 + p lives at m[g, p, c*128:(c+1)*128]."""
    m4 = np.asarray(m).reshape(NGRP, P, 16, OUT_DIM)
    return np.ascontiguousarray(m4.transpose(0, 2, 1, 3)).reshape(EPC_PAD, OUT_DIM)


def _install_ntff_hook():
    """Best-effort: register the axon NTFF profile hook when the image's
    antenv package lacks axon_hooks (needed only for trace=True runs)."""
    import sys
    import types

    try:
        from antenv.axon_hooks import get_axon_ntff_profile_hook  # noqa: F401

        return
    except ImportError:
        pass
    try:
        from trn_agent_boot.trn_boot import _ntff_profile_via_ctypes

        hook = _ntff_profile_via_ctypes("/opt/axon/libaxon_pjrt.so")
        mod = types.ModuleType("antenv.axon_hooks")
        mod._hook = hook
        mod.get_axon_ntff_profile_hook = lambda: mod._hook
        mod.set_axon_ntff_profile_hook = lambda h: setattr(mod, "_hook", h)
        sys.modules["antenv.axon_hooks"] = mod
        import antenv

        antenv.axon_hooks = mod
    except Exception:
        pass


def kernel(h, e, edge_index, W_e, W_hu, W_hw):
    nc = get_nc()
    in_maps, pos_list = _prep_in_maps(h, e, edge_index, W_e, W_hu, W_hw)
    trace = bool(int(os.environ.get("KERNEL_TRACE", "0")))
    if trace:
        _install_ntff_hook()
    res = run_bass_kernel_spmd(nc, in_maps, list(range(NCORES)), trace=trace)
    LAST["exec_time_ns"] = res.exec_time_ns
    LAST["results"] = res
    out = np.empty((N_EDGES, OUT_DIM), dtype=np.float32)
    for c in range(NCORES):
        flat = _unscramble(res.results[c]["msgs"])
        out[c * EPC : (c + 1) * EPC] = flat[pos_list[c]].astype(np.float32)
    return out


# revision 22
# speedup vs baseline: 1.5544x; 1.0113x over previous
"""Trainium2 Bass kernel for GNN message passing:

    messages = e @ W_e.T + (h @ W_hu.T)[src] + (h @ W_hw.T)[tgt]

Strategy (8 NeuronCores, edge-parallel, bf16, raw-bass manual pipeline):
  - Edges sharded 100k per core; h and weights replicated. All device
    math in bf16 (harness gate is 2e-2; this lands ~4e-3).
  - Phase 1: project the node table once into an internal-DRAM combined
    table hub[n] = [hu[n] | hw[n]] (bf16, 512B rows, so table-store DMAs
    move 512B contiguous segments at full rate). The lo half (25088
    rows) is written first so phase-2 gathers of lo/lo edges start while
    the hi half is still being built.
  - Phase 2: per 2048-edge group, hu[src] / hw[tgt] are fetched with
    non-transpose DMAGatherAnt (elem 256B, elem_step 512B into the two
    column halves of hub), spread over 4 SWDGE queues -- measured ~3.6x
    the 1-queue random-gather rate, and non-transpose mode is the only
    multi-queue-safe mode (concurrent transpose-mode gathers corrupt
    each other through the shared per-engine transpose path). The tensor
    engine computes ee = e.T @ W_e.T into PSUM (edges on partitions);
    DVE does mt = psum + hu_g + hw_g; one DMA stores each group.
  - Raw engine blocks with manual per-slot semaphores (the Tile
    scheduler's SWDGE-sem round-robin cannot express 4-queue gathers).
  - int16 gather indices cover 32k rows -> hub addressed as lo/hi halves
    (split 25088); the host buckets each core's edges by
    (src-half, tgt-half) into 4 fixed-capacity segments so the SPMD
    program is uniform, and un-permutes the output.
"""
import os
from contextlib import ExitStack

import numpy as np
import ml_dtypes

import concourse.bass as bass
from concourse import bacc, mybir
from concourse.bass_utils import run_bass_kernel_spmd
from concourse.library_config import mlp

N_NODES = 50000
N_EDGES = 800000
IN_DIM = 128
OUT_DIM = 128
EDGE_DIM = 64
NCORES = 8

P = 128
NODES_PAD = 50176
HALF = 25088                # int16 index range split
NBLK = NODES_PAD // 512     # 98 phase-1 blocks of 512 nodes
LOBLK = HALF // 512         # 49 blocks cover the lo half

EPC = N_EDGES // NCORES     # 100000 edges per core
G_EDGES = 2048              # edges per gather group
CAPS = [13, 13, 13, 13]     # groups per bucket; mean 25000 +- 137, cap 26624
NGRP = sum(CAPS)            # 52
EPC_PAD = NGRP * G_EDGES    # 106496
SEG_EDGE_START = [0]
for _c in CAPS:
    SEG_EDGE_START.append(SEG_EDGE_START[-1] + _c * G_EDGES)

HBUF = 3                    # phase-1 h-block buffers
GBUF = 12                   # gather group buffers (even: slot sems stay on
                            # one queue parity)
EBUF = 4                    # e-tile buffers
MBUF = 4                    # output tile buffers
BPS = 4                     # phase-1 blocks per table store

F32 = mybir.dt.float32
BF16 = mybir.dt.bfloat16
I16 = mybir.dt.int16
NPBF16 = ml_dtypes.bfloat16

_CACHE = {}
LAST = {}


def _seg_of(G):
    s = 0
    while G >= sum(CAPS[: s + 1]):
        s += 1
    return s


def _build():
    nc = bacc.Bacc(
        "TRN2",
        target_bir_lowering=False,
        debug=False,
        enable_asserts=True,
        num_devices=NCORES,
        num_swdge_queues=4,
    )

    hT = nc.dram_tensor("hT", [P, NODES_PAD], BF16, kind="ExternalInput")
    # cols 0:256 = [W_hu.T | W_hw.T]; cols 256:384 = W_e.T stacked twice
    wall = nc.dram_tensor("wall", [P, 384], BF16, kind="ExternalInput")
    eP = nc.dram_tensor("eP", [NGRP, P, G_EDGES // 2], BF16, kind="ExternalInput")
    sidx = nc.dram_tensor("sidx", [P, NGRP * (G_EDGES // 16)], I16, kind="ExternalInput")
    tidx = nc.dram_tensor("tidx", [P, NGRP * (G_EDGES // 16)], I16, kind="ExternalInput")
    msgs = nc.dram_tensor("msgs", [NGRP, P, G_EDGES], BF16, kind="ExternalOutput")
    hub = nc.dram_tensor("hub", [NODES_PAD, 2 * OUT_DIM], BF16)

    with (
        nc.Block(no_gpsimd_drain=True) as block,
        nc.sbuf_tensor("wall_t", [P, 384], BF16) as wall_t,
        nc.sbuf_tensor("sidx_t", [P, NGRP * (G_EDGES // 16)], I16) as sidx_t,
        nc.sbuf_tensor("tidx_t", [P, NGRP * (G_EDGES // 16)], I16) as tidx_t,
        ExitStack() as ctx,
    ):
        hb = [
            ctx.enter_context(nc.sbuf_tensor(f"hb{i}", [P, 512], BF16))
            for i in range(HBUF)
        ]
        ot = [
            ctx.enter_context(nc.sbuf_tensor(f"ot{i}", [P, BPS * 1024], BF16))
            for i in range(2)
        ]
        eb = [
            ctx.enter_context(nc.sbuf_tensor(f"eb{i}", [P, G_EDGES // 2], BF16))
            for i in range(EBUF)
        ]
        hug = [
            ctx.enter_context(nc.sbuf_tensor(f"hug{i}", [P, 16, OUT_DIM], BF16))
            for i in range(GBUF)
        ]
        hwg = [
            ctx.enter_context(nc.sbuf_tensor(f"hwg{i}", [P, 16, OUT_DIM], BF16))
            for i in range(GBUF)
        ]
        mtb = [
            ctx.enter_context(nc.sbuf_tensor(f"mt{i}", [P, G_EDGES], BF16))
            for i in range(MBUF)
        ]
        ps1 = [
            ctx.enter_context(nc.psum_tensor(f"ps1_{i}", [P, 512], F32))
            for i in range(4)
        ]
        ps2 = [
            ctx.enter_context(nc.psum_tensor(f"ps2_{i}", [P, 1024], F32))
            for i in range(2)
        ]
        s_ld = ctx.enter_context(nc.semaphore("s_ld"))
        s_hb = [ctx.enter_context(nc.semaphore(f"s_hb{i}")) for i in range(HBUF)]
        s_p1 = ctx.enter_context(nc.semaphore("s_p1"))
        s_c1d = ctx.enter_context(nc.semaphore("s_c1d"))
        s_c1a = ctx.enter_context(nc.semaphore("s_c1a"))
        s_t = [ctx.enter_context(nc.semaphore(f"s_t{i}")) for i in range(2)]
        s_eb = [ctx.enter_context(nc.semaphore(f"s_eb{i}")) for i in range(EBUF)]
        s_hs = [ctx.enter_context(nc.semaphore(f"s_hs{i}")) for i in range(GBUF)]
        s_ht = [ctx.enter_context(nc.semaphore(f"s_ht{i}")) for i in range(GBUF)]
        s_mm = ctx.enter_context(nc.semaphore("s_mm"))
        s_a1 = ctx.enter_context(nc.semaphore("s_a1"))
        s_add = ctx.enter_context(nc.semaphore("s_add"))
        s_st = [ctx.enter_context(nc.semaphore(f"s_st{i}")) for i in range(MBUF)]

        # store s covers blocks [s*BPS, (s+1)*BPS); lo rows need blocks 0..48
        # -> stores 0..12; full table -> stores 0..24 (NBLK=98, BPS=4 -> 25)
        NST = NBLK // BPS + (1 if NBLK % BPS else 0)
        LO_ST = (LOBLK + BPS - 1) // BPS  # 13
        LO_T = [16 * ((LO_ST - 1 - k) // 2 + 1) for k in range(2)]
        FULL_T = [16 * ((NST - 1 - k) // 2 + 1) for k in range(2)]

        @block.sync
        def _(sp: bass.BassEngine):
            sp.dma_start(wall_t[:, :], wall[:, :]).then_inc(s_ld, 16)
            sp.dma_start(sidx_t[:, :], sidx[:, :]).then_inc(s_ld, 16)
            sp.dma_start(tidx_t[:, :], tidx[:, :]).then_inc(s_ld, 16)
            for G in range(NGRP):
                sp.wait_ge(s_add, 2 * G + 2)
                sp.dma_start(msgs[G], mtb[G % MBUF][:, :]).then_inc(
                    s_st[G % MBUF], 16
                )
            for k in range(MBUF):
                sp.wait_ge(s_st[k], 16 * ((NGRP - 1 - k) // MBUF + 1))

        @block.scalar
        def _(act: bass.BassScalarEngine):
            for i in range(EBUF):
                act.dma_start(eb[i][:, :], eP[i]).then_inc(s_eb[i], 16)
            for i in range(HBUF):
                act.dma_start(hb[i][:, :], hT[:, i * 512 : (i + 1) * 512]).then_inc(
                    s_hb[i], 16
                )
            for b in range(NBLK):
                act.wait_ge(s_p1, 2 * b + 2)
                st = b // BPS
                if st >= 2 and b % BPS == 0:
                    act.wait_ge(s_t[st % 2], 16 * (st // 2))
                off = (b % BPS) * 1024
                act.copy(
                    out=ot[st % 2][:, off + 512 : off + 1024],
                    in_=ps1[(2 * b + 1) % 4][:, :],
                ).then_inc(s_c1a, 1)
                if b % BPS == BPS - 1 or b == NBLK - 1:
                    blo, bhi = st * BPS, min((st + 1) * BPS, NBLK)
                    act.wait_ge(s_c1d, bhi)
                    # self-edge: our own h1 copies must drain before the DMA
                    # engines read ot
                    act.wait_ge(s_c1a, bhi)
                    act.dma_start(
                        hub[blo * 512 : bhi * 512, :].rearrange(
                            "(s p) d -> p s d", p=P
                        ),
                        ot[st % 2][:, : (bhi - blo) * 1024].rearrange(
                            "p (s d) -> p s d", d=2 * OUT_DIM
                        ),
                    ).then_inc(s_t[st % 2], 16)
                if b + HBUF < NBLK:
                    act.dma_start(
                        hb[(b + HBUF) % HBUF][:, :],
                        hT[:, (b + HBUF) * 512 : (b + HBUF + 1) * 512],
                    ).then_inc(s_hb[(b + HBUF) % HBUF], 16)
            for G in range(NGRP):
                if G + EBUF < NGRP:
                    act.wait_ge(s_mm, 2 * G + 2)
                    act.dma_start(
                        eb[(G + EBUF) % EBUF][:, :], eP[G + EBUF]
                    ).then_inc(s_eb[(G + EBUF) % EBUF], 16)

        @block.vector
        def _(dve: bass.BassVectorEngine):
            for b in range(NBLK):
                dve.wait_ge(s_p1, 2 * b + 1)
                st = b // BPS
                if st >= 2 and b % BPS == 0:
                    dve.wait_ge(s_t[st % 2], 16 * (st // 2))
                off = (b % BPS) * 1024
                dve.tensor_copy(
                    out=ot[st % 2][:, off : off + 512], in_=ps1[(2 * b) % 4][:, :]
                ).then_inc(s_c1d, 1)
            def add1(G):
                hu_t = hug[G % GBUF][:, :, :].rearrange("p c d -> p (c d)")
                dve.wait_ge(s_hs[G % GBUF], 16 * (G // GBUF + 1))
                if G >= MBUF:
                    dve.wait_ge(s_st[G % MBUF], 16 * ((G - MBUF) // MBUF + 1))
                mt = mtb[G % MBUF]
                dve.wait_ge(s_mm, 2 * G + 1)
                dve.tensor_add(
                    out=mt[:, 0:1024], in0=ps2[0][:, :], in1=hu_t[:, 0:1024]
                )
                dve.wait_ge(s_mm, 2 * G + 2)
                dve.tensor_add(
                    out=mt[:, 1024:2048], in0=ps2[1][:, :], in1=hu_t[:, 1024:2048]
                ).then_inc(s_a1, 1)

            def add2(G):
                # reading mt back: add1(G)'s writes must have drained; its
                # s_a1 inc fired G+1, and we run inside add1(G+1)'s slot so
                # this wait is normally already satisfied
                hw_t = hwg[G % GBUF][:, :, :].rearrange("p c d -> p (c d)")
                dve.wait_ge(s_ht[G % GBUF], 16 * (G // GBUF + 1))
                dve.wait_ge(s_a1, G + 1)
                mt = mtb[G % MBUF]
                dve.tensor_add(
                    out=mt[:, 0:1024], in0=mt[:, 0:1024], in1=hw_t[:, 0:1024]
                )
                dve.tensor_add(
                    out=mt[:, 1024:2048],
                    in0=mt[:, 1024:2048],
                    in1=hw_t[:, 1024:2048],
                ).then_inc(s_add, 2)

            for G in range(NGRP):
                add1(G)
                if G >= 1:
                    add2(G - 1)
            add2(NGRP - 1)

        @block.gpsimd
        def _(gp: bass.BassGpSimd):
            gp.load_library(mlp)
            gp.wait_ge(s_ld, 48)
            for G in range(NGRP):
                seg = _seg_of(G)
                if G == 0:
                    gp.wait_ge(s_t[0], LO_T[0])
                    gp.wait_ge(s_t[1], LO_T[1])
                elif G == CAPS[0]:
                    gp.wait_ge(s_t[0], FULL_T[0])
                    gp.wait_ge(s_t[1], FULL_T[1])
                hu_src = (
                    hub[0:HALF, 0:OUT_DIM]
                    if seg < 2
                    else hub[HALF:NODES_PAD, 0:OUT_DIM]
                )
                hw_src = (
                    hub[0:HALF, OUT_DIM : 2 * OUT_DIM]
                    if seg % 2 == 0
                    else hub[HALF:NODES_PAD, OUT_DIM : 2 * OUT_DIM]
                )
                if G >= GBUF:
                    gp.wait_ge(s_add, 2 * (G - GBUF) + 2)
                gp.dma_gather(
                    hug[G % GBUF][:, :, :],
                    hu_src,
                    sidx_t[:, G * 128 : (G + 1) * 128],
                    G_EDGES,
                    G_EDGES,
                    OUT_DIM,
                    elem_step=2 * OUT_DIM,
                    single_packet=False,
                    queue_num=2 * (G % 2),
                ).then_inc(s_hs[G % GBUF], 16)
                gp.dma_gather(
                    hwg[G % GBUF][:, :, :],
                    hw_src,
                    tidx_t[:, G * 128 : (G + 1) * 128],
                    G_EDGES,
                    G_EDGES,
                    OUT_DIM,
                    elem_step=2 * OUT_DIM,
                    single_packet=False,
                    queue_num=2 * (G % 2) + 1,
                ).then_inc(s_ht[G % GBUF], 16)

        @block.tensor
        def _(pe: bass.BassTensorEngine):
            pe.wait_ge(s_ld, 48)
            for b in range(NBLK):
                pe.wait_ge(s_hb[b % HBUF], 16 * (b // HBUF + 1))
                if b >= 2:
                    pe.wait_ge(s_c1d, b - 1)
                    pe.wait_ge(s_c1a, b - 1)
                for h in range(2):
                    ps = ps1[(2 * b + h) % 4]
                    for s in range(2):
                        mm = pe.matmul(
                            out=ps[:, s * 256 : (s + 1) * 256],
                            lhsT=hb[b % HBUF][:, (2 * h + s) * P : (2 * h + s + 1) * P],
                            rhs=wall_t[:, 0:256],
                            start=True,
                            stop=True,
                        )
                    mm.then_inc(s_p1, 1)
            for G in range(NGRP):
                pe.wait_ge(s_eb[G % EBUF], 16 * (G // EBUF + 1))
                if G >= 1:
                    pe.wait_ge(s_a1, G)
                for h in range(2):
                    pb = 0 if h == 0 else 64
                    for t in range(8):
                        mm = pe.matmul(
                            out=ps2[h][:, t * P : (t + 1) * P],
                            lhsT=eb[G % EBUF][pb : pb + 64, t * P : (t + 1) * P],
                            rhs=wall_t[pb : pb + 64, 256:384],
                            start=True,
                            stop=True,
                        )
                    mm.then_inc(s_mm, 1)

    nc.compile()
    return nc


def get_nc():
    if "nc" not in _CACHE:
        _CACHE["nc"] = _build()
    return _CACHE["nc"]


def _prep_in_maps(h, e, edge_index, W_e, W_hu, W_hw):
    """Returns (in_maps, pos_list): pos_list[c][i] = padded-edge slot of
    core c holding original edge c*EPC+i (slot = g*2048 + c*128 + p)."""
    h = np.asarray(h, dtype=np.float32)
    e = np.asarray(e, dtype=np.float32)
    src = np.asarray(edge_index[0]).astype(np.int64)
    tgt = np.asarray(edge_index[1]).astype(np.int64)
    W_e = np.asarray(W_e, dtype=np.float32)
    W_hu = np.asarray(W_hu, dtype=np.float32)
    W_hw = np.asarray(W_hw, dtype=np.float32)

    hT = np.zeros((P, NODES_PAD), dtype=NPBF16)
    hT[:, :N_NODES] = h.astype(NPBF16).T

    wall = np.concatenate(
        [W_hu.T, W_hw.T, np.vstack([W_e.T, W_e.T])], axis=1
    ).astype(NPBF16)

    in_maps = []
    pos_list = []
    for c in range(NCORES):
        sl = slice(c * EPC, (c + 1) * EPC)
        sc, tc_, ec = src[sl], tgt[sl], e[sl]
        bucket = 2 * (sc >= HALF).astype(np.int64) + (tc_ >= HALF).astype(np.int64)

        e_pad = np.zeros((EPC_PAD, EDGE_DIM), dtype=np.float32)
        s16 = np.zeros((EPC_PAD,), dtype=np.int16)
        t16 = np.zeros((EPC_PAD,), dtype=np.int16)
        pos = np.empty((EPC,), dtype=np.int64)
        for b in range(4):
            selb = np.flatnonzero(bucket == b)
            if len(selb) > CAPS[b] * G_EDGES:
                raise RuntimeError(
                    f"bucket {b} overflow on core {c}: {len(selb)} > {CAPS[b] * G_EDGES}"
                )
            base = SEG_EDGE_START[b]
            pos[selb] = base + np.arange(len(selb))
            e_pad[base : base + len(selb)] = ec[selb]
            s16[base : base + len(selb)] = (sc[selb] - HALF * (b >> 1)).astype(np.int16)
            t16[base : base + len(selb)] = (tc_[selb] - HALF * (b & 1)).astype(np.int16)

        ePc = np.ascontiguousarray(
            e_pad.reshape(NGRP, 2, G_EDGES // 2, EDGE_DIM)
            .astype(NPBF16)
            .transpose(0, 1, 3, 2)
        ).reshape(NGRP, P, G_EDGES // 2)

        # dma_gather index layout: value j of group g sits at
        # [j % 16, g*128 + j//16], replicated across the 8 gpsimd banks.
        def idx_layout(v16):
            a16 = v16.reshape(NGRP, G_EDGES // 16, 16).transpose(2, 0, 1).reshape(
                16, NGRP * (G_EDGES // 16)
            )
            return np.ascontiguousarray(np.tile(a16, (8, 1)))

        in_maps.append(
            {
                "hT": hT,
                "wall": wall,
                "eP": ePc,
                "sidx": idx_layout(s16),
                "tidx": idx_layout(t16),
            }
        )
        pos_list.append(pos)
    return in_maps, pos_list


def _unscramble(m):
    """[NGRP, P, G_EDGES] device layout -> [EPC_PAD, OUT_DIM]; edge slot
    g*2048 + c*128 + p lives at m[g, p, c*128:(c+1)*128]."""
    m4 = np.asarray(m).reshape(NGRP, P, 16, OUT_DIM)
    return np.ascontiguousarray(m4.transpose(0, 2, 1, 3)).reshape(EPC_PAD, OUT_DIM)


def _install_ntff_hook():
    """Best-effort: register the axon NTFF profile hook when the image's
    antenv package lacks axon_hooks (needed only for trace=True runs)."""
    import sys
    import types

    try:
        from antenv.axon_hooks import get_axon_ntff_profile_hook  # noqa: F401

        return
    except ImportError:
        pass
    try:
        from trn_agent_boot.trn_boot import _ntff_profile_via_ctypes

        hook = _ntff_profile_via_ctypes("/opt/axon/libaxon_pjrt.so")
        mod = types.ModuleType("antenv.axon_hooks")
        mod._hook = hook
        mod.get_axon_ntff_profile_hook = lambda: mod._hook
        mod.set_axon_ntff_profile_hook = lambda h: setattr(mod, "_hook", h)
        sys.modules["antenv.axon_hooks"] = mod
        import antenv

        antenv.axon_hooks = mod
    except Exception:
        pass


def kernel(h, e, edge_index, W_e, W_hu, W_hw):
    nc = get_nc()
    in_maps, pos_list = _prep_in_maps(h, e, edge_index, W_e, W_hu, W_hw)
    trace = bool(int(os.environ.get("KERNEL_TRACE", "0")))
    if trace:
        _install_ntff_hook()
    res = run_bass_kernel_spmd(nc, in_maps, list(range(NCORES)), trace=trace)
    LAST["exec_time_ns"] = res.exec_time_ns
    LAST["results"] = res
    out = np.empty((N_EDGES, OUT_DIM), dtype=np.float32)
    for c in range(NCORES):
        flat = _unscramble(res.results[c]["msgs"])
        out[c * EPC : (c + 1) * EPC] = flat[pos_list[c]].astype(np.float32)
    return out


# revision 24
# speedup vs baseline: 1.6434x; 1.0572x over previous
"""Trainium2 Bass kernel for GNN message passing:

    messages = e @ W_e.T + (h @ W_hu.T)[src] + (h @ W_hw.T)[tgt]

Strategy (8 NeuronCores, edge-parallel, bf16, raw-bass manual pipeline):
  - Edges sharded 100k per core; h and weights replicated. All device
    math in bf16 (harness gate is 2e-2; this lands ~4e-3).
  - Phase 1: project the node table once into an internal-DRAM combined
    table hub[n] = [hu[n] | hw[n]] (bf16, 512B rows, so table-store DMAs
    move 512B contiguous segments at full rate). The lo half (25088
    rows) is written first so phase-2 gathers of lo/lo edges start while
    the hi half is still being built.
  - Phase 2: per 2048-edge group, hu[src] / hw[tgt] are fetched with
    non-transpose DMAGatherAnt (elem 256B, elem_step 512B into the two
    column halves of hub), spread over 4 SWDGE queues -- measured ~3.6x
    the 1-queue random-gather rate, and non-transpose mode is the only
    multi-queue-safe mode (concurrent transpose-mode gathers corrupt
    each other through the shared per-engine transpose path). The tensor
    engine computes ee = e.T @ W_e.T into PSUM (edges on partitions);
    DVE does mt = psum + hu_g + hw_g; one DMA stores each group.
  - Raw engine blocks with manual per-slot semaphores (the Tile
    scheduler's SWDGE-sem round-robin cannot express 4-queue gathers).
  - int16 gather indices cover 32k rows -> hub addressed as lo/hi halves
    (split 25088); the host buckets each core's edges by
    (src-half, tgt-half) into 4 fixed-capacity segments so the SPMD
    program is uniform, and un-permutes the output.
"""
import os
from contextlib import ExitStack

import numpy as np
import ml_dtypes

import concourse.bass as bass
from concourse import bacc, mybir
from concourse.bass_utils import run_bass_kernel_spmd
from concourse.library_config import mlp

N_NODES = 50000
N_EDGES = 800000
IN_DIM = 128
OUT_DIM = 128
EDGE_DIM = 64
NCORES = 8

P = 128
NODES_PAD = 50176
HALF = 25088                # int16 index range split
NBLK = NODES_PAD // 512     # 98 phase-1 blocks of 512 nodes
LOBLK = HALF // 512         # 49 blocks cover the lo half

EPC = N_EDGES // NCORES     # 100000 edges per core
G_EDGES = 2048              # edges per gather group
CAPS = [13, 13, 13, 13]     # groups per bucket; mean 25000 +- 137, cap 26624
NGRP = sum(CAPS)            # 52
EPC_PAD = NGRP * G_EDGES    # 106496
SEG_EDGE_START = [0]
for _c in CAPS:
    SEG_EDGE_START.append(SEG_EDGE_START[-1] + _c * G_EDGES)

HBUF = 3                    # phase-1 h-block buffers
GBUF = 12                   # gather group buffers (even: slot sems stay on
                            # one queue parity)
EBUF = 4                    # e-tile buffers
MBUF = 4                    # output tile buffers
BPS = 4                     # phase-1 blocks per table store

F32 = mybir.dt.float32
BF16 = mybir.dt.bfloat16
I16 = mybir.dt.int16
NPBF16 = ml_dtypes.bfloat16

_CACHE = {}
LAST = {}


def _seg_of(G):
    s = 0
    while G >= sum(CAPS[: s + 1]):
        s += 1
    return s


def _build():
    nc = bacc.Bacc(
        "TRN2",
        target_bir_lowering=False,
        debug=False,
        enable_asserts=True,
        num_devices=NCORES,
        num_swdge_queues=4,
    )

    hT = nc.dram_tensor("hT", [P, NODES_PAD], BF16, kind="ExternalInput")
    # cols 0:256 = [W_hu.T | W_hw.T]; cols 256:384 = W_e.T stacked twice
    wall = nc.dram_tensor("wall", [P, 384], BF16, kind="ExternalInput")
    eP = nc.dram_tensor("eP", [NGRP, P, G_EDGES // 2], BF16, kind="ExternalInput")
    sidx = nc.dram_tensor("sidx", [P, NGRP * (G_EDGES // 16)], I16, kind="ExternalInput")
    tidx = nc.dram_tensor("tidx", [P, NGRP * (G_EDGES // 16)], I16, kind="ExternalInput")
    msgs = nc.dram_tensor("msgs", [NGRP, P, G_EDGES], BF16, kind="ExternalOutput")
    hub = nc.dram_tensor("hub", [NODES_PAD, 2 * OUT_DIM], BF16)

    with (
        nc.Block(no_gpsimd_drain=True) as block,
        nc.sbuf_tensor("wall_t", [P, 384], BF16) as wall_t,
        nc.sbuf_tensor("sidx_t", [P, NGRP * (G_EDGES // 16)], I16) as sidx_t,
        nc.sbuf_tensor("tidx_t", [P, NGRP * (G_EDGES // 16)], I16) as tidx_t,
        ExitStack() as ctx,
    ):
        hb = [
            ctx.enter_context(nc.sbuf_tensor(f"hb{i}", [P, 512], BF16))
            for i in range(HBUF)
        ]
        ot = [
            ctx.enter_context(nc.sbuf_tensor(f"ot{i}", [P, BPS * 1024], BF16))
            for i in range(2)
        ]
        eb = [
            ctx.enter_context(nc.sbuf_tensor(f"eb{i}", [P, G_EDGES // 2], BF16))
            for i in range(EBUF)
        ]
        hug = [
            ctx.enter_context(nc.sbuf_tensor(f"hug{i}", [P, 16, OUT_DIM], BF16))
            for i in range(GBUF)
        ]
        hwg = [
            ctx.enter_context(nc.sbuf_tensor(f"hwg{i}", [P, 16, OUT_DIM], BF16))
            for i in range(GBUF)
        ]
        mtb = [
            ctx.enter_context(nc.sbuf_tensor(f"mt{i}", [P, G_EDGES], BF16))
            for i in range(MBUF)
        ]
        ps1 = [
            ctx.enter_context(nc.psum_tensor(f"ps1_{i}", [P, 512], F32))
            for i in range(4)
        ]
        ps2 = [
            ctx.enter_context(nc.psum_tensor(f"ps2_{i}", [P, 1024], F32))
            for i in range(2)
        ]
        s_ld = ctx.enter_context(nc.semaphore("s_ld"))
        s_hb = [ctx.enter_context(nc.semaphore(f"s_hb{i}")) for i in range(HBUF)]
        s_p1 = ctx.enter_context(nc.semaphore("s_p1"))
        s_c1d = ctx.enter_context(nc.semaphore("s_c1d"))
        s_c1a = ctx.enter_context(nc.semaphore("s_c1a"))
        s_t = [ctx.enter_context(nc.semaphore(f"s_t{i}")) for i in range(2)]
        s_eb = [ctx.enter_context(nc.semaphore(f"s_eb{i}")) for i in range(EBUF)]
        s_hs = [ctx.enter_context(nc.semaphore(f"s_hs{i}")) for i in range(GBUF)]
        s_ht = [ctx.enter_context(nc.semaphore(f"s_ht{i}")) for i in range(GBUF)]
        s_mm = ctx.enter_context(nc.semaphore("s_mm"))
        s_a1 = ctx.enter_context(nc.semaphore("s_a1"))
        s_add = ctx.enter_context(nc.semaphore("s_add"))
        s_st = [ctx.enter_context(nc.semaphore(f"s_st{i}")) for i in range(MBUF)]

        # store s covers blocks [s*BPS, (s+1)*BPS); lo rows need blocks 0..48
        # -> stores 0..12; full table -> stores 0..24 (NBLK=98, BPS=4 -> 25)
        NST = NBLK // BPS + (1 if NBLK % BPS else 0)
        LO_ST = (LOBLK + BPS - 1) // BPS  # 13
        LO_T = [16 * ((LO_ST - 1 - k) // 2 + 1) for k in range(2)]
        FULL_T = [16 * ((NST - 1 - k) // 2 + 1) for k in range(2)]

        @block.sync
        def _(sp: bass.BassEngine):
            sp.dma_start(wall_t[:, :], wall[:, :]).then_inc(s_ld, 16)
            sp.dma_start(sidx_t[:, :], sidx[:, :]).then_inc(s_ld, 16)
            sp.dma_start(tidx_t[:, :], tidx[:, :]).then_inc(s_ld, 16)

            def tstore(st):
                blo, bhi = st * BPS, min((st + 1) * BPS, NBLK)
                sp.wait_ge(s_c1d, bhi)
                sp.wait_ge(s_c1a, bhi)
                sp.dma_start(
                    hub[blo * 512 : bhi * 512, :].rearrange(
                        "(s p) d -> p s d", p=P
                    ),
                    ot[st % 2][:, : (bhi - blo) * 1024].rearrange(
                        "p (s d) -> p s d", d=2 * OUT_DIM
                    ),
                ).then_inc(s_t[st % 2], 16)

            def mstore(G):
                sp.wait_ge(s_add, 2 * G + 2)
                sp.dma_start(msgs[G], mtb[G % MBUF][:, :]).then_inc(
                    s_st[G % MBUF], 16
                )

            NST_ = NBLK // BPS + (1 if NBLK % BPS else 0)
            LO_ST_ = (LOBLK + BPS - 1) // BPS
            for st in range(LO_ST_):
                tstore(st)
            G = 0
            for st in range(LO_ST_, NST_):
                tstore(st)
                if G < NGRP:
                    mstore(G)
                    G += 1
            while G < NGRP:
                mstore(G)
                G += 1
            for k in range(MBUF):
                sp.wait_ge(s_st[k], 16 * ((NGRP - 1 - k) // MBUF + 1))

        @block.scalar
        def _(act: bass.BassScalarEngine):
            for i in range(EBUF):
                act.dma_start(eb[i][:, :], eP[i]).then_inc(s_eb[i], 16)
            for i in range(HBUF):
                act.dma_start(hb[i][:, :], hT[:, i * 512 : (i + 1) * 512]).then_inc(
                    s_hb[i], 16
                )
            def a_copy(b):
                act.wait_ge(s_p1, 2 * b + 2)
                st = b // BPS
                if st >= 2 and b % BPS == 0:
                    act.wait_ge(s_t[st % 2], 16 * (st // 2))
                off = (b % BPS) * 1024
                act.copy(
                    out=ot[st % 2][:, off + 512 : off + 1024],
                    in_=ps1[(2 * b + 1) % 4][:, :],
                ).then_inc(s_c1a, 1)
                if b + HBUF < NBLK:
                    act.dma_start(
                        hb[(b + HBUF) % HBUF][:, :],
                        hT[:, (b + HBUF) * 512 : (b + HBUF + 1) * 512],
                    ).then_inc(s_hb[(b + HBUF) % HBUF], 16)

            def a_eb(G):
                if G + EBUF < NGRP:
                    act.wait_ge(s_mm, 2 * G + 2)
                    act.dma_start(
                        eb[(G + EBUF) % EBUF][:, :], eP[G + EBUF]
                    ).then_inc(s_eb[(G + EBUF) % EBUF], 16)

            for b in range(LOBLK):
                a_copy(b)
            b = LOBLK
            for G in range(CAPS[0]):
                for _ in range(4):
                    if b < NBLK:
                        a_copy(b)
                        b += 1
                a_eb(G)
            while b < NBLK:
                a_copy(b)
                b += 1
            for G in range(CAPS[0], NGRP):
                a_eb(G)

        @block.vector
        def _(dve: bass.BassVectorEngine):
            def d_copy(b):
                dve.wait_ge(s_p1, 2 * b + 1)
                st = b // BPS
                if st >= 2 and b % BPS == 0:
                    dve.wait_ge(s_t[st % 2], 16 * (st // 2))
                off = (b % BPS) * 1024
                dve.tensor_copy(
                    out=ot[st % 2][:, off : off + 512], in_=ps1[(2 * b) % 4][:, :]
                ).then_inc(s_c1d, 1)

            def add1(G):
                hu_t = hug[G % GBUF][:, :, :].rearrange("p c d -> p (c d)")
                dve.wait_ge(s_hs[G % GBUF], 16 * (G // GBUF + 1))
                if G >= MBUF:
                    dve.wait_ge(s_st[G % MBUF], 16 * ((G - MBUF) // MBUF + 1))
                mt = mtb[G % MBUF]
                dve.wait_ge(s_mm, 2 * G + 1)
                dve.tensor_add(
                    out=mt[:, 0:1024], in0=ps2[0][:, :], in1=hu_t[:, 0:1024]
                )
                dve.wait_ge(s_mm, 2 * G + 2)
                dve.tensor_add(
                    out=mt[:, 1024:2048], in0=ps2[1][:, :], in1=hu_t[:, 1024:2048]
                ).then_inc(s_a1, 1)

            def add2(G):
                # reading mt back: add1(G)'s writes must have drained; its
                # s_a1 inc fired G+1, and we run inside add1(G+1)'s slot so
                # this wait is normally already satisfied
                hw_t = hwg[G % GBUF][:, :, :].rearrange("p c d -> p (c d)")
                dve.wait_ge(s_ht[G % GBUF], 16 * (G // GBUF + 1))
                dve.wait_ge(s_a1, G + 1)
                mt = mtb[G % MBUF]
                dve.tensor_add(
                    out=mt[:, 0:1024], in0=mt[:, 0:1024], in1=hw_t[:, 0:1024]
                )
                dve.tensor_add(
                    out=mt[:, 1024:2048],
                    in0=mt[:, 1024:2048],
                    in1=hw_t[:, 1024:2048],
                ).then_inc(s_add, 2)

            for b in range(LOBLK):
                d_copy(b)
            b = LOBLK
            for G in range(CAPS[0]):
                for _ in range(4):
                    if b < NBLK:
                        d_copy(b)
                        b += 1
                add1(G)
                if G >= 1:
                    add2(G - 1)
            while b < NBLK:
                d_copy(b)
                b += 1
            for G in range(CAPS[0], NGRP):
                add1(G)
                add2(G - 1)
            add2(NGRP - 1)

        @block.gpsimd
        def _(gp: bass.BassGpSimd):
            gp.load_library(mlp)
            gp.wait_ge(s_ld, 48)
            for G in range(NGRP):
                seg = _seg_of(G)
                if G == 0:
                    gp.wait_ge(s_t[0], LO_T[0])
                    gp.wait_ge(s_t[1], LO_T[1])
                elif G == CAPS[0]:
                    gp.wait_ge(s_t[0], FULL_T[0])
                    gp.wait_ge(s_t[1], FULL_T[1])
                hu_src = (
                    hub[0:HALF, 0:OUT_DIM]
                    if seg < 2
                    else hub[HALF:NODES_PAD, 0:OUT_DIM]
                )
                hw_src = (
                    hub[0:HALF, OUT_DIM : 2 * OUT_DIM]
                    if seg % 2 == 0
                    else hub[HALF:NODES_PAD, OUT_DIM : 2 * OUT_DIM]
                )
                if G >= GBUF:
                    gp.wait_ge(s_add, 2 * (G - GBUF) + 2)
                gp.dma_gather(
                    hug[G % GBUF][:, :, :],
                    hu_src,
                    sidx_t[:, G * 128 : (G + 1) * 128],
                    G_EDGES,
                    G_EDGES,
                    OUT_DIM,
                    elem_step=2 * OUT_DIM,
                    single_packet=False,
                    queue_num=2 * (G % 2),
                ).then_inc(s_hs[G % GBUF], 16)
                gp.dma_gather(
                    hwg[G % GBUF][:, :, :],
                    hw_src,
                    tidx_t[:, G * 128 : (G + 1) * 128],
                    G_EDGES,
                    G_EDGES,
                    OUT_DIM,
                    elem_step=2 * OUT_DIM,
                    single_packet=False,
                    queue_num=2 * (G % 2) + 1,
                ).then_inc(s_ht[G % GBUF], 16)

        @block.tensor
        def _(pe: bass.BassTensorEngine):
            pe.wait_ge(s_ld, 48)

            def p_blk(b):
                pe.wait_ge(s_hb[b % HBUF], 16 * (b // HBUF + 1))
                if b >= 2:
                    pe.wait_ge(s_c1d, b - 1)
                    pe.wait_ge(s_c1a, b - 1)
                for h in range(2):
                    ps = ps1[(2 * b + h) % 4]
                    for s in range(2):
                        mm = pe.matmul(
                            out=ps[:, s * 256 : (s + 1) * 256],
                            lhsT=hb[b % HBUF][:, (2 * h + s) * P : (2 * h + s + 1) * P],
                            rhs=wall_t[:, 0:256],
                            start=True,
                            stop=True,
                        )
                    mm.then_inc(s_p1, 1)

            def p_grp(G):
                pe.wait_ge(s_eb[G % EBUF], 16 * (G // EBUF + 1))
                if G >= 1:
                    pe.wait_ge(s_a1, G)
                for h in range(2):
                    pb = 0 if h == 0 else 64
                    for t in range(8):
                        mm = pe.matmul(
                            out=ps2[h][:, t * P : (t + 1) * P],
                            lhsT=eb[G % EBUF][pb : pb + 64, t * P : (t + 1) * P],
                            rhs=wall_t[pb : pb + 64, 256:384],
                            start=True,
                            stop=True,
                        )
                    mm.then_inc(s_mm, 1)

            for b in range(LOBLK):
                p_blk(b)
            b = LOBLK
            for G in range(CAPS[0]):
                p_grp(G)
                for _ in range(4):
                    if b < NBLK:
                        p_blk(b)
                        b += 1
            while b < NBLK:
                p_blk(b)
                b += 1
            for G in range(CAPS[0], NGRP):
                p_grp(G)

    nc.compile()
    return nc


def get_nc():
    if "nc" not in _CACHE:
        _CACHE["nc"] = _build()
    return _CACHE["nc"]


def _prep_in_maps(h, e, edge_index, W_e, W_hu, W_hw):
    """Returns (in_maps, pos_list): pos_list[c][i] = padded-edge slot of
    core c holding original edge c*EPC+i (slot = g*2048 + c*128 + p)."""
    h = np.asarray(h, dtype=np.float32)
    e = np.asarray(e, dtype=np.float32)
    src = np.asarray(edge_index[0]).astype(np.int64)
    tgt = np.asarray(edge_index[1]).astype(np.int64)
    W_e = np.asarray(W_e, dtype=np.float32)
    W_hu = np.asarray(W_hu, dtype=np.float32)
    W_hw = np.asarray(W_hw, dtype=np.float32)

    hT = np.zeros((P, NODES_PAD), dtype=NPBF16)
    hT[:, :N_NODES] = h.astype(NPBF16).T

    wall = np.concatenate(
        [W_hu.T, W_hw.T, np.vstack([W_e.T, W_e.T])], axis=1
    ).astype(NPBF16)

    in_maps = []
    pos_list = []
    for c in range(NCORES):
        sl = slice(c * EPC, (c + 1) * EPC)
        sc, tc_, ec = src[sl], tgt[sl], e[sl]
        bucket = 2 * (sc >= HALF).astype(np.int64) + (tc_ >= HALF).astype(np.int64)

        e_pad = np.zeros((EPC_PAD, EDGE_DIM), dtype=np.float32)
        s16 = np.zeros((EPC_PAD,), dtype=np.int16)
        t16 = np.zeros((EPC_PAD,), dtype=np.int16)
        pos = np.empty((EPC,), dtype=np.int64)
        for b in range(4):
            selb = np.flatnonzero(bucket == b)
            if len(selb) > CAPS[b] * G_EDGES:
                raise RuntimeError(
                    f"bucket {b} overflow on core {c}: {len(selb)} > {CAPS[b] * G_EDGES}"
                )
            base = SEG_EDGE_START[b]
            pos[selb] = base + np.arange(len(selb))
            e_pad[base : base + len(selb)] = ec[selb]
            s16[base : base + len(selb)] = (sc[selb] - HALF * (b >> 1)).astype(np.int16)
            t16[base : base + len(selb)] = (tc_[selb] - HALF * (b & 1)).astype(np.int16)

        ePc = np.ascontiguousarray(
            e_pad.reshape(NGRP, 2, G_EDGES // 2, EDGE_DIM)
            .astype(NPBF16)
            .transpose(0, 1, 3, 2)
        ).reshape(NGRP, P, G_EDGES // 2)

        # dma_gather index layout: value j of group g sits at
        # [j % 16, g*128 + j//16], replicated across the 8 gpsimd banks.
        def idx_layout(v16):
            a16 = v16.reshape(NGRP, G_EDGES // 16, 16).transpose(2, 0, 1).reshape(
                16, NGRP * (G_EDGES // 16)
            )
            return np.ascontiguousarray(np.tile(a16, (8, 1)))

        in_maps.append(
            {
                "hT": hT,
                "wall": wall,
                "eP": ePc,
                "sidx": idx_layout(s16),
                "tidx": idx_layout(t16),
            }
        )
        pos_list.append(pos)
    return in_maps, pos_list


def _unscramble(m):
    """[NGRP, P, G_EDGES] device layout -> [EPC_PAD, OUT_DIM]; edge slot
    g*2048 + c*128 + p lives at m[g, p, c*128:(c+1)*128]."""
    m4 = np.asarray(m).reshape(NGRP, P, 16, OUT_DIM)
    return np.ascontiguousarray(m4.transpose(0, 2, 1, 3)).reshape(EPC_PAD, OUT_DIM)


def _install_ntff_hook():
    """Best-effort: register the axon NTFF profile hook when the image's
    antenv package lacks axon_hooks (needed only for trace=True runs)."""
    import sys
    import types

    try:
        from antenv.axon_hooks import get_axon_ntff_profile_hook  # noqa: F401

        return
    except ImportError:
        pass
    try:
        from trn_agent_boot.trn_boot import _ntff_profile_via_ctypes

        hook = _ntff_profile_via_ctypes("/opt/axon/libaxon_pjrt.so")
        mod = types.ModuleType("antenv.axon_hooks")
        mod._hook = hook
        mod.get_axon_ntff_profile_hook = lambda: mod._hook
        mod.set_axon_ntff_profile_hook = lambda h: setattr(mod, "_hook", h)
        sys.modules["antenv.axon_hooks"] = mod
        import antenv

        antenv.axon_hooks = mod
    except Exception:
        pass


def kernel(h, e, edge_index, W_e, W_hu, W_hw):
    nc = get_nc()
    in_maps, pos_list = _prep_in_maps(h, e, edge_index, W_e, W_hu, W_hw)
    trace = bool(int(os.environ.get("KERNEL_TRACE", "0")))
    if trace:
        _install_ntff_hook()
    res = run_bass_kernel_spmd(nc, in_maps, list(range(NCORES)), trace=trace)
    LAST["exec_time_ns"] = res.exec_time_ns
    LAST["results"] = res
    out = np.empty((N_EDGES, OUT_DIM), dtype=np.float32)
    for c in range(NCORES):
        flat = _unscramble(res.results[c]["msgs"])
        out[c * EPC : (c + 1) * EPC] = flat[pos_list[c]].astype(np.float32)
    return out


# revision 26
# speedup vs baseline: 1.7011x; 1.0351x over previous
"""Trainium2 Bass kernel for GNN message passing:

    messages = e @ W_e.T + (h @ W_hu.T)[src] + (h @ W_hw.T)[tgt]

Strategy (8 NeuronCores, edge-parallel, bf16, raw-bass manual pipeline):
  - Edges sharded 100k per core; h and weights replicated. All device
    math in bf16 (harness gate is 2e-2; this lands ~4e-3).
  - Phase 1: project the node table once into an internal-DRAM combined
    table hub[n] = [hu[n] | hw[n]] (bf16, 512B rows, so table-store DMAs
    move 512B contiguous segments at full rate). The lo half (25088
    rows) is written first so phase-2 gathers of lo/lo edges start while
    the hi half is still being built.
  - Phase 2: per 2048-edge group, hu[src] / hw[tgt] are fetched with
    non-transpose DMAGatherAnt (elem 256B, elem_step 512B into the two
    column halves of hub), spread over 4 SWDGE queues -- measured ~3.6x
    the 1-queue random-gather rate, and non-transpose mode is the only
    multi-queue-safe mode (concurrent transpose-mode gathers corrupt
    each other through the shared per-engine transpose path). The tensor
    engine computes ee = e.T @ W_e.T into PSUM (edges on partitions);
    DVE does mt = psum + hu_g + hw_g; one DMA stores each group.
  - Raw engine blocks with manual per-slot semaphores (the Tile
    scheduler's SWDGE-sem round-robin cannot express 4-queue gathers).
  - int16 gather indices cover 32k rows -> hub addressed as lo/hi halves
    (split 25088); the host buckets each core's edges by
    (src-half, tgt-half) into 4 fixed-capacity segments so the SPMD
    program is uniform, and un-permutes the output.
"""
import os
from contextlib import ExitStack

import numpy as np
import ml_dtypes

import concourse.bass as bass
from concourse import bacc, mybir
from concourse.bass_utils import run_bass_kernel_spmd
from concourse.library_config import mlp

N_NODES = 50000
N_EDGES = 800000
IN_DIM = 128
OUT_DIM = 128
EDGE_DIM = 64
NCORES = 8

P = 128
NODES_PAD = 50176
HALF = 30720                # int16 index range split (< 32768)
NBLK = NODES_PAD // 512     # 98 phase-1 blocks of 512 nodes
LOBLK = HALF // 512         # 49 blocks cover the lo half

EPC = N_EDGES // NCORES     # 100000 edges per core
G_EDGES = 2048              # edges per gather group
CAPS = [19, 12, 12, 8]      # groups per bucket (asymmetric split: ll is
                            # 37.7% of edges -> more gather work available
                            # while the hi table half is still being built)
NGRP = sum(CAPS)            # 52
EPC_PAD = NGRP * G_EDGES    # 106496
SEG_EDGE_START = [0]
for _c in CAPS:
    SEG_EDGE_START.append(SEG_EDGE_START[-1] + _c * G_EDGES)

HBUF = 3                    # phase-1 h-block buffers
GBUF = 8                    # gather group buffers (even: slot sems stay on
                            # one queue parity)
EBUF = 4                    # e-tile buffers
MBUF = 4                    # output tile buffers
BPS = 4                     # phase-1 blocks per table store

F32 = mybir.dt.float32
BF16 = mybir.dt.bfloat16
I16 = mybir.dt.int16
NPBF16 = ml_dtypes.bfloat16

_CACHE = {}
LAST = {}


def _seg_of(G):
    s = 0
    while G >= sum(CAPS[: s + 1]):
        s += 1
    return s


def _build():
    nc = bacc.Bacc(
        "TRN2",
        target_bir_lowering=False,
        debug=False,
        enable_asserts=True,
        num_devices=NCORES,
        num_swdge_queues=4,
    )

    hT = nc.dram_tensor("hT", [P, NODES_PAD], BF16, kind="ExternalInput")
    # cols 0:256 = [W_hu.T | W_hw.T]; cols 256:384 = W_e.T stacked twice
    wall = nc.dram_tensor("wall", [P, 384], BF16, kind="ExternalInput")
    eP = nc.dram_tensor("eP", [NGRP, P, G_EDGES // 2], BF16, kind="ExternalInput")
    sidx = nc.dram_tensor("sidx", [P, NGRP * (G_EDGES // 16)], I16, kind="ExternalInput")
    tidx = nc.dram_tensor("tidx", [P, NGRP * (G_EDGES // 16)], I16, kind="ExternalInput")
    msgs = nc.dram_tensor("msgs", [NGRP, P, G_EDGES], BF16, kind="ExternalOutput")
    hub = nc.dram_tensor("hub", [NODES_PAD, 2 * OUT_DIM], BF16)

    with (
        nc.Block(no_gpsimd_drain=True) as block,
        nc.sbuf_tensor("wall_t", [P, 384], BF16) as wall_t,
        nc.sbuf_tensor("sidx_t", [P, NGRP * (G_EDGES // 16)], I16) as sidx_t,
        nc.sbuf_tensor("tidx_t", [P, NGRP * (G_EDGES // 16)], I16) as tidx_t,
        ExitStack() as ctx,
    ):
        hb = [
            ctx.enter_context(nc.sbuf_tensor(f"hb{i}", [P, 512], BF16))
            for i in range(HBUF)
        ]
        ot = [
            ctx.enter_context(nc.sbuf_tensor(f"ot{i}", [P, BPS * 1024], BF16))
            for i in range(2)
        ]
        eb = [
            ctx.enter_context(nc.sbuf_tensor(f"eb{i}", [P, G_EDGES // 2], BF16))
            for i in range(EBUF)
        ]
        hug = [
            ctx.enter_context(nc.sbuf_tensor(f"hug{i}", [P, 16, OUT_DIM], BF16))
            for i in range(GBUF)
        ]
        hwg = [
            ctx.enter_context(nc.sbuf_tensor(f"hwg{i}", [P, 16, OUT_DIM], BF16))
            for i in range(GBUF)
        ]
        mtb = [
            ctx.enter_context(nc.sbuf_tensor(f"mt{i}", [P, G_EDGES], BF16))
            for i in range(MBUF)
        ]
        ps1 = [
            ctx.enter_context(nc.psum_tensor(f"ps1_{i}", [P, 512], F32))
            for i in range(4)
        ]
        ps2 = [
            ctx.enter_context(nc.psum_tensor(f"ps2_{i}", [P, 1024], F32))
            for i in range(2)
        ]
        s_ld = ctx.enter_context(nc.semaphore("s_ld"))
        s_hb = [ctx.enter_context(nc.semaphore(f"s_hb{i}")) for i in range(HBUF)]
        s_p1 = ctx.enter_context(nc.semaphore("s_p1"))
        s_c1d = ctx.enter_context(nc.semaphore("s_c1d"))
        s_c1a = ctx.enter_context(nc.semaphore("s_c1a"))
        s_t = [ctx.enter_context(nc.semaphore(f"s_t{i}")) for i in range(2)]
        s_eb = [ctx.enter_context(nc.semaphore(f"s_eb{i}")) for i in range(EBUF)]
        s_hs = [ctx.enter_context(nc.semaphore(f"s_hs{i}")) for i in range(GBUF)]
        s_ht = [ctx.enter_context(nc.semaphore(f"s_ht{i}")) for i in range(GBUF)]
        s_mm = ctx.enter_context(nc.semaphore("s_mm"))
        s_a1 = ctx.enter_context(nc.semaphore("s_a1"))
        s_add = ctx.enter_context(nc.semaphore("s_add"))
        s_st = [ctx.enter_context(nc.semaphore(f"s_st{i}")) for i in range(MBUF)]

        # store s covers blocks [s*BPS, (s+1)*BPS); lo rows need blocks 0..48
        # -> stores 0..12; full table -> stores 0..24 (NBLK=98, BPS=4 -> 25)
        NST = NBLK // BPS + (1 if NBLK % BPS else 0)
        LO_ST = (LOBLK + BPS - 1) // BPS  # 13
        LO_T = [16 * ((LO_ST - 1 - k) // 2 + 1) for k in range(2)]
        FULL_T = [16 * ((NST - 1 - k) // 2 + 1) for k in range(2)]

        @block.sync
        def _(sp: bass.BassEngine):
            sp.dma_start(wall_t[:, :], wall[:, :]).then_inc(s_ld, 16)
            sp.dma_start(sidx_t[:, :], sidx[:, :]).then_inc(s_ld, 16)
            sp.dma_start(tidx_t[:, :], tidx[:, :]).then_inc(s_ld, 16)

            def tstore(st):
                blo, bhi = st * BPS, min((st + 1) * BPS, NBLK)
                sp.wait_ge(s_c1d, bhi)
                sp.wait_ge(s_c1a, bhi)
                sp.dma_start(
                    hub[blo * 512 : bhi * 512, :].rearrange(
                        "(s p) d -> p s d", p=P
                    ),
                    ot[st % 2][:, : (bhi - blo) * 1024].rearrange(
                        "p (s d) -> p s d", d=2 * OUT_DIM
                    ),
                ).then_inc(s_t[st % 2], 16)

            def mstore(G):
                sp.wait_ge(s_add, 2 * G + 2)
                sp.dma_start(msgs[G], mtb[G % MBUF][:, :]).then_inc(
                    s_st[G % MBUF], 16
                )

            NST_ = NBLK // BPS + (1 if NBLK % BPS else 0)
            LO_ST_ = (LOBLK + BPS - 1) // BPS
            for st in range(LO_ST_):
                tstore(st)
            # tstore(LO_ST_+j) transitively needs add1(2j+1), which needs
            # mstore(2j+1-MBUF) -- emit msg stores it depends on first
            G = 0
            for j in range(NST_ - LO_ST_):
                while G <= 2 * j + 1 - MBUF and G < NGRP:
                    mstore(G)
                    G += 1
                tstore(LO_ST_ + j)
            while G < NGRP:
                mstore(G)
                G += 1
            for k in range(MBUF):
                sp.wait_ge(s_st[k], 16 * ((NGRP - 1 - k) // MBUF + 1))

        @block.scalar
        def _(act: bass.BassScalarEngine):
            for i in range(EBUF):
                act.dma_start(eb[i][:, :], eP[i]).then_inc(s_eb[i], 16)
            for i in range(HBUF):
                act.dma_start(hb[i][:, :], hT[:, i * 512 : (i + 1) * 512]).then_inc(
                    s_hb[i], 16
                )
            def a_copy(b):
                act.wait_ge(s_p1, 2 * b + 2)
                st = b // BPS
                if st >= 2 and b % BPS == 0:
                    act.wait_ge(s_t[st % 2], 16 * (st // 2))
                off = (b % BPS) * 1024
                act.copy(
                    out=ot[st % 2][:, off + 512 : off + 1024],
                    in_=ps1[(2 * b + 1) % 4][:, :],
                ).then_inc(s_c1a, 1)
                if b + HBUF < NBLK:
                    act.dma_start(
                        hb[(b + HBUF) % HBUF][:, :],
                        hT[:, (b + HBUF) * 512 : (b + HBUF + 1) * 512],
                    ).then_inc(s_hb[(b + HBUF) % HBUF], 16)

            def a_eb(G):
                if G + EBUF < NGRP:
                    act.wait_ge(s_mm, 2 * G + 2)
                    act.dma_start(
                        eb[(G + EBUF) % EBUF][:, :], eP[G + EBUF]
                    ).then_inc(s_eb[(G + EBUF) % EBUF], 16)

            for b in range(LOBLK):
                a_copy(b)
            b = LOBLK
            for G in range(CAPS[0]):
                for _ in range(2):
                    if b < NBLK:
                        a_copy(b)
                        b += 1
                a_eb(G)
            while b < NBLK:
                a_copy(b)
                b += 1
            for G in range(CAPS[0], NGRP):
                a_eb(G)

        @block.vector
        def _(dve: bass.BassVectorEngine):
            def d_copy(b):
                dve.wait_ge(s_p1, 2 * b + 1)
                st = b // BPS
                if st >= 2 and b % BPS == 0:
                    dve.wait_ge(s_t[st % 2], 16 * (st // 2))
                off = (b % BPS) * 1024
                dve.tensor_copy(
                    out=ot[st % 2][:, off : off + 512], in_=ps1[(2 * b) % 4][:, :]
                ).then_inc(s_c1d, 1)

            def add1(G):
                hu_t = hug[G % GBUF][:, :, :].rearrange("p c d -> p (c d)")
                dve.wait_ge(s_hs[G % GBUF], 16 * (G // GBUF + 1))
                if G >= MBUF:
                    dve.wait_ge(s_st[G % MBUF], 16 * ((G - MBUF) // MBUF + 1))
                mt = mtb[G % MBUF]
                dve.wait_ge(s_mm, 2 * G + 1)
                dve.tensor_add(
                    out=mt[:, 0:1024], in0=ps2[0][:, :], in1=hu_t[:, 0:1024]
                )
                dve.wait_ge(s_mm, 2 * G + 2)
                dve.tensor_add(
                    out=mt[:, 1024:2048], in0=ps2[1][:, :], in1=hu_t[:, 1024:2048]
                ).then_inc(s_a1, 1)

            def add2(G):
                # reading mt back: add1(G)'s writes must have drained; its
                # s_a1 inc fired G+1, and we run inside add1(G+1)'s slot so
                # this wait is normally already satisfied
                hw_t = hwg[G % GBUF][:, :, :].rearrange("p c d -> p (c d)")
                dve.wait_ge(s_ht[G % GBUF], 16 * (G // GBUF + 1))
                dve.wait_ge(s_a1, G + 1)
                mt = mtb[G % MBUF]
                dve.tensor_add(
                    out=mt[:, 0:1024], in0=mt[:, 0:1024], in1=hw_t[:, 0:1024]
                )
                dve.tensor_add(
                    out=mt[:, 1024:2048],
                    in0=mt[:, 1024:2048],
                    in1=hw_t[:, 1024:2048],
                ).then_inc(s_add, 2)

            for b in range(LOBLK):
                d_copy(b)
            b = LOBLK
            for G in range(CAPS[0]):
                for _ in range(2):
                    if b < NBLK:
                        d_copy(b)
                        b += 1
                add1(G)
                if G >= 1:
                    add2(G - 1)
            while b < NBLK:
                d_copy(b)
                b += 1
            for G in range(CAPS[0], NGRP):
                add1(G)
                add2(G - 1)
            add2(NGRP - 1)

        @block.gpsimd
        def _(gp: bass.BassGpSimd):
            gp.load_library(mlp)
            gp.wait_ge(s_ld, 48)
            for G in range(NGRP):
                seg = _seg_of(G)
                if G == 0:
                    gp.wait_ge(s_t[0], LO_T[0])
                    gp.wait_ge(s_t[1], LO_T[1])
                elif G == CAPS[0]:
                    gp.wait_ge(s_t[0], FULL_T[0])
                    gp.wait_ge(s_t[1], FULL_T[1])
                hu_src = (
                    hub[0:HALF, 0:OUT_DIM]
                    if seg < 2
                    else hub[HALF:NODES_PAD, 0:OUT_DIM]
                )
                hw_src = (
                    hub[0:HALF, OUT_DIM : 2 * OUT_DIM]
                    if seg % 2 == 0
                    else hub[HALF:NODES_PAD, OUT_DIM : 2 * OUT_DIM]
                )
                if G >= GBUF:
                    gp.wait_ge(s_add, 2 * (G - GBUF) + 2)
                gp.dma_gather(
                    hug[G % GBUF][:, :, :],
                    hu_src,
                    sidx_t[:, G * 128 : (G + 1) * 128],
                    G_EDGES,
                    G_EDGES,
                    OUT_DIM,
                    elem_step=2 * OUT_DIM,
                    single_packet=False,
                    queue_num=2 * (G % 2),
                ).then_inc(s_hs[G % GBUF], 16)
                gp.dma_gather(
                    hwg[G % GBUF][:, :, :],
                    hw_src,
                    tidx_t[:, G * 128 : (G + 1) * 128],
                    G_EDGES,
                    G_EDGES,
                    OUT_DIM,
                    elem_step=2 * OUT_DIM,
                    single_packet=False,
                    queue_num=2 * (G % 2) + 1,
                ).then_inc(s_ht[G % GBUF], 16)

        @block.tensor
        def _(pe: bass.BassTensorEngine):
            pe.wait_ge(s_ld, 48)

            def p_blk(b):
                pe.wait_ge(s_hb[b % HBUF], 16 * (b // HBUF + 1))
                if b >= 2:
                    pe.wait_ge(s_c1d, b - 1)
                    pe.wait_ge(s_c1a, b - 1)
                for h in range(2):
                    ps = ps1[(2 * b + h) % 4]
                    for s in range(2):
                        mm = pe.matmul(
                            out=ps[:, s * 256 : (s + 1) * 256],
                            lhsT=hb[b % HBUF][:, (2 * h + s) * P : (2 * h + s + 1) * P],
                            rhs=wall_t[:, 0:256],
                            start=True,
                            stop=True,
                        )
                    mm.then_inc(s_p1, 1)

            def p_grp(G):
                pe.wait_ge(s_eb[G % EBUF], 16 * (G // EBUF + 1))
                if G >= 1:
                    pe.wait_ge(s_a1, G)
                for h in range(2):
                    pb = 0 if h == 0 else 64
                    for t in range(8):
                        mm = pe.matmul(
                            out=ps2[h][:, t * P : (t + 1) * P],
                            lhsT=eb[G % EBUF][pb : pb + 64, t * P : (t + 1) * P],
                            rhs=wall_t[pb : pb + 64, 256:384],
                            start=True,
                            stop=True,
                        )
                    mm.then_inc(s_mm, 1)

            for b in range(LOBLK):
                p_blk(b)
            b = LOBLK
            for G in range(CAPS[0]):
                p_grp(G)
                for _ in range(2):
                    if b < NBLK:
                        p_blk(b)
                        b += 1
            while b < NBLK:
                p_blk(b)
                b += 1
            for G in range(CAPS[0], NGRP):
                p_grp(G)

    nc.compile()
    return nc


def get_nc():
    if "nc" not in _CACHE:
        _CACHE["nc"] = _build()
    return _CACHE["nc"]


def _prep_in_maps(h, e, edge_index, W_e, W_hu, W_hw):
    """Returns (in_maps, pos_list): pos_list[c][i] = padded-edge slot of
    core c holding original edge c*EPC+i (slot = g*2048 + c*128 + p)."""
    h = np.asarray(h, dtype=np.float32)
    e = np.asarray(e, dtype=np.float32)
    src = np.asarray(edge_index[0]).astype(np.int64)
    tgt = np.asarray(edge_index[1]).astype(np.int64)
    W_e = np.asarray(W_e, dtype=np.float32)
    W_hu = np.asarray(W_hu, dtype=np.float32)
    W_hw = np.asarray(W_hw, dtype=np.float32)

    hT = np.zeros((P, NODES_PAD), dtype=NPBF16)
    hT[:, :N_NODES] = h.astype(NPBF16).T

    wall = np.concatenate(
        [W_hu.T, W_hw.T, np.vstack([W_e.T, W_e.T])], axis=1
    ).astype(NPBF16)

    in_maps = []
    pos_list = []
    for c in range(NCORES):
        sl = slice(c * EPC, (c + 1) * EPC)
        sc, tc_, ec = src[sl], tgt[sl], e[sl]
        bucket = 2 * (sc >= HALF).astype(np.int64) + (tc_ >= HALF).astype(np.int64)

        e_pad = np.zeros((EPC_PAD, EDGE_DIM), dtype=np.float32)
        s16 = np.zeros((EPC_PAD,), dtype=np.int16)
        t16 = np.zeros((EPC_PAD,), dtype=np.int16)
        pos = np.empty((EPC,), dtype=np.int64)
        for b in range(4):
            selb = np.flatnonzero(bucket == b)
            if len(selb) > CAPS[b] * G_EDGES:
                raise RuntimeError(
                    f"bucket {b} overflow on core {c}: {len(selb)} > {CAPS[b] * G_EDGES}"
                )
            base = SEG_EDGE_START[b]
            pos[selb] = base + np.arange(len(selb))
            e_pad[base : base + len(selb)] = ec[selb]
            s16[base : base + len(selb)] = (sc[selb] - HALF * (b >> 1)).astype(np.int16)
            t16[base : base + len(selb)] = (tc_[selb] - HALF * (b & 1)).astype(np.int16)

        ePc = np.ascontiguousarray(
            e_pad.reshape(NGRP, 2, G_EDGES // 2, EDGE_DIM)
            .astype(NPBF16)
            .transpose(0, 1, 3, 2)
        ).reshape(NGRP, P, G_EDGES // 2)

        # dma_gather index layout: value j of group g sits at
        # [j % 16, g*128 + j//16], replicated across the 8 gpsimd banks.
        def idx_layout(v16):
            a16 = v16.reshape(NGRP, G_EDGES // 16, 16).transpose(2, 0, 1).reshape(
                16, NGRP * (G_EDGES // 16)
            )
            return np.ascontiguousarray(np.tile(a16, (8, 1)))

        in_maps.append(
            {
                "hT": hT,
                "wall": wall,
                "eP": ePc,
                "sidx": idx_layout(s16),
                "tidx": idx_layout(t16),
            }
        )
        pos_list.append(pos)
    return in_maps, pos_list


def _unscramble(m):
    """[NGRP, P, G_EDGES] device layout -> [EPC_PAD, OUT_DIM]; edge slot
    g*2048 + c*128 + p lives at m[g, p, c*128:(c+1)*128]."""
    m4 = np.asarray(m).reshape(NGRP, P, 16, OUT_DIM)
    return np.ascontiguousarray(m4.transpose(0, 2, 1, 3)).reshape(EPC_PAD, OUT_DIM)


def _install_ntff_hook():
    """Best-effort: register the axon NTFF profile hook when the image's
    antenv package lacks axon_hooks (needed only for trace=True runs)."""
    import sys
    import types

    try:
        from antenv.axon_hooks import get_axon_ntff_profile_hook  # noqa: F401

        return
    except ImportError:
        pass
    try:
        from trn_agent_boot.trn_boot import _ntff_profile_via_ctypes

        hook = _ntff_profile_via_ctypes("/opt/axon/libaxon_pjrt.so")
        mod = types.ModuleType("antenv.axon_hooks")
        mod._hook = hook
        mod.get_axon_ntff_profile_hook = lambda: mod._hook
        mod.set_axon_ntff_profile_hook = lambda h: setattr(mod, "_hook", h)
        sys.modules["antenv.axon_hooks"] = mod
        import antenv

        antenv.axon_hooks = mod
    except Exception:
        pass


def kernel(h, e, edge_index, W_e, W_hu, W_hw):
    nc = get_nc()
    in_maps, pos_list = _prep_in_maps(h, e, edge_index, W_e, W_hu, W_hw)
    trace = bool(int(os.environ.get("KERNEL_TRACE", "0")))
    if trace:
        _install_ntff_hook()
    res = run_bass_kernel_spmd(nc, in_maps, list(range(NCORES)), trace=trace)
    LAST["exec_time_ns"] = res.exec_time_ns
    LAST["results"] = res
    out = np.empty((N_EDGES, OUT_DIM), dtype=np.float32)
    for c in range(NCORES):
        flat = _unscramble(res.results[c]["msgs"])
        out[c * EPC : (c + 1) * EPC] = flat[pos_list[c]].astype(np.float32)
    return out
